# revision 1
# baseline (speedup 1.0000x reference)
"""Trainium2 Bass kernel for MineralDepositGCN (3x GCNConv+BN + MLP head).

Strategy (8 NeuronCores, SPMD single program):
  - Shard nodes by source range: core c owns nodes [c*12500, (c+1)*12500),
    padded to 12800 per core (node n -> padded id 12800*(n//12500)+n%12500).
  - Edges assigned to the core owning src: gather of h[src] is device-local.
  - Per layer: project own shard h@W -> local bf16 gather table [12800, 128]
    (cols 0:64 valid, 64:128 junk pad so rows are 256B for dma_gather);
    dma_gather per-edge rows; one-hot (built on-device from dst ids via
    tensor_scalar is_equal*ew, built once on layer 0 and cached in DRAM --
    the graph is layer-invariant) x messages matmuls segment-sum into PSUM per
    512-dst supertile; f32 partials ReduceScatter(add) across cores so each
    core ends with the aggregate for its own node range; bias+relu+BN with
    AllReduce'd global stats (pad rows contribute exactly relu(bias), which
    is subtracted in closed form).
  - MLP head computed feature-major on each core's shard; host reassembles.
"""
import os
import numpy as np
import ml_dtypes

from concourse import bass, bacc, tile, mybir
from concourse import bass_utils
from concourse.bass_interp import get_hw_module

BF16 = mybir.dt.bfloat16
F32 = mybir.dt.float32
I16 = mybir.dt.int16
ALU = mybir.AluOpType
ACTF = mybir.ActivationFunctionType

NCORES = 8
EPS = 1e-5


def _cfg(n_nodes, in_c, hid, ncls):
    shard = n_nodes // NCORES
    npad = ((shard + 511) // 512) * 512
    return dict(
        N=n_nodes, IN_C=in_c, HID=hid, NCLS=ncls,
        SHARD=shard, NPAD=npad,
        NTILES=npad // 128,            # 128-node dst tiles per core
        NST=npad // 512,               # 512-node supertiles per core
        NST_ALL=(npad // 512) * NCORES,
        NTOT=npad * NCORES,
    )


def _preprocess(x, edge_index, edge_attr, cfg):
    """Host-side sharding: returns per-core input dicts + chunk count C."""
    N, SHARD, NPAD = cfg["N"], cfg["SHARD"], cfg["NPAD"]
    src = edge_index[0].astype(np.int64)
    dst = edge_index[1].astype(np.int64)
    ew = np.asarray(edge_attr, dtype=np.float32)

    owner = src // SHARD
    np.minimum(owner, NCORES - 1, out=owner)   # guard (src < N always)
    local_src = src - owner * SHARD
    # padded global dst id
    dstp = (dst // SHARD) * NPAD + (dst % SHARD)
    gtile = dstp // 128                         # global 128-dst tile id
    NT_ALL = cfg["NTILES"] * NCORES

    # per (core, tile) counts -> C
    counts = np.zeros((NCORES, NT_ALL), dtype=np.int64)
    flat = owner * NT_ALL + gtile
    np.add.at(counts.reshape(-1), flat, 1)
    C = int(max(1, -(-counts.max() // 128)))
    SLOT_T = 128 * C
    NSLOT = NT_ALL * SLOT_T
    NCHUNK = NT_ALL * C

    per_core = []
    for c in range(NCORES):
        m = owner == c
        ls = local_src[m].astype(np.int64)
        dp = dstp[m]
        w = ew[m]
        gt = gtile[m]
        order = np.argsort(gt, kind="stable")
        ls, dp, w, gt = ls[order], dp[order], w[order], gt[order]
        cnt = counts[c]
        # slot position: tile base + rank within tile
        starts = np.zeros(NT_ALL, dtype=np.int64)
        starts[1:] = np.cumsum(cnt)[:-1]
        rank = np.arange(ls.shape[0], dtype=np.int64) - starts[gt]
        slot = gt * SLOT_T + rank

        g_idx = np.zeros(NSLOT, dtype=np.int16)          # pad -> row 0
        o_dst = np.full(NSLOT, 255.0, dtype=np.float32)  # pad -> no match
        o_ew = np.zeros(NSLOT, dtype=np.float32)
        g_idx[slot] = ls.astype(np.int16)
        o_dst[slot] = (dp - gt * 128).astype(np.float32)
        o_ew[slot] = w

        # dma_gather wrapped index layout, per gather group of GS slots
        GS = cfg["GS"]
        ng = NSLOT // GS
        wrapped = g_idx.reshape(ng, GS // 16, 16).transpose(0, 2, 1)  # [ng,16,GS/16]
        wrapped = wrapped.reshape(ng * 16, GS // 16)
        # -> tensor [128, NSLOT//16]: group g occupies cols [g*GS/16,(g+1)*GS/16)
        idx_t = np.zeros((128, NSLOT // 16), dtype=np.int16)
        for g in range(ng):
            blk = wrapped[g * 16:(g + 1) * 16]            # [16, GS/16]
            idx_t[:, g * (GS // 16):(g + 1) * (GS // 16)] = np.tile(blk, (8, 1))

        per_core.append(dict(
            g_idx=idx_t,
            dst_rel=o_dst.reshape(NCHUNK, 128).T.copy(),
            ew_s=o_ew.reshape(NCHUNK, 128).T.copy(),
        ))

    # x transposed + padded, bf16
    for c in range(NCORES):
        xs = np.zeros((cfg["IN_C"], NPAD), dtype=np.float32)
        xs[:, :SHARD] = np.asarray(x[c * SHARD:(c + 1) * SHARD]).T
        per_core[c]["x_t"] = xs.astype(ml_dtypes.bfloat16)
    return per_core, C, NCHUNK, NSLOT


def _build(cfg, C, NCHUNK, NSLOT, GST):
    IN_C, HID, NCLS = cfg["IN_C"], cfg["HID"], cfg["NCLS"]
    NPAD, NTILES, NST, NST_ALL = (cfg["NPAD"], cfg["NTILES"], cfg["NST"],
                                  cfg["NST_ALL"])
    NPT = NPAD // 128
    GS = cfg["GS"]
    NG = NSLOT // GS                     # gather groups (whole layer)
    CPG = GS // 128                      # chunks per group = GST*4*C
    NPADDING = float(NCORES * NPAD - cfg["N"])
    INVN = 1.0 / cfg["N"]

    nc = bacc.Bacc("TRN2", target_bir_lowering=False, debug=False,
                   num_devices=NCORES)

    def din(name, shape, dt):
        return nc.dram_tensor(name, shape, dt, kind="ExternalInput").ap()

    x_t_d = din("x_t", [IN_C, NPAD], BF16)
    gidx_d = din("g_idx", [128, NSLOT // 16], I16)
    dst_d = din("dst_rel", [128, NCHUNK], F32)
    ew_d = din("ew_s", [128, NCHUNK], F32)
    iota_d = din("iota128", [128, 128], BF16)
    cw_d = [din(f"conv_w{l}", [IN_C if l == 0 else HID, HID], BF16)
            for l in range(3)]
    cb_d = [din(f"conv_b{l}", [HID, 1], F32) for l in range(3)]
    bng_d = [din(f"bn_g{l}", [HID, 1], F32) for l in range(3)]
    bnb_d = [din(f"bn_be{l}", [HID, 1], F32) for l in range(3)]
    mw1_d = din("mlp_w1", [HID, 2 * HID], BF16)
    mw2_d = din("mlp_w2", [2 * HID, HID], BF16)
    mw3_d = din("mlp_w3", [HID, NCLS], BF16)
    mb1_d = din("mlp_b1", [2 * HID, 1], F32)
    mb2_d = din("mlp_b2", [HID, 1], F32)
    mb3_d = din("mlp_b3", [NCLS, 1], F32)
    out_d = nc.dram_tensor("out5", [NCLS, NPAD], F32, kind="ExternalOutput").ap()
    dbg = bool(os.environ.get("KERNEL_DEBUG"))
    if dbg:
        dbg_tab = nc.dram_tensor("dbg_tab", [NPAD, 128], BF16,
                                 kind="ExternalOutput").ap()
        dbg_raw = nc.dram_tensor("dbg_raw", [HID, NPAD], F32,
                                 kind="ExternalOutput").ap()
        dbg_h = nc.dram_tensor("dbg_h", [HID, NPAD], F32,
                               kind="ExternalOutput").ap()

    rg = [list(range(NCORES))]
    SKIP_GATHER = bool(os.environ.get("KSKIP_GATHER"))
    SKIP_CC = bool(os.environ.get("KSKIP_CC"))
    SKIP_CHUNKS = bool(os.environ.get("KSKIP_CHUNKS"))
    SKIP_PROJ = bool(os.environ.get("KSKIP_PROJ"))

    with tile.TileContext(nc) as tc:
        with tc.tile_pool(name="sb", bufs=1) as sb, \
             tc.tile_pool(name="sb2", bufs=2) as sb2, \
             tc.tile_pool(name="sb4", bufs=6) as sb4, \
             tc.tile_pool(name="ps", bufs=2, space="PSUM") as ps, \
             tc.tile_pool(name="ps1", bufs=1, space="PSUM") as ps1, \
             tc.tile_pool(name="dram", bufs=1, space="DRAM") as dram, \
             tc.tile_pool(name="dram2", bufs=2, space="DRAM") as dram2:

            # ---- persistent loads ----
            iota_t = sb.tile([128, 128], BF16, tag="iota")
            nc.sync.dma_start(out=iota_t[:], in_=iota_d[:])
            dst_t = sb.tile([128, NCHUNK], F32, tag="dst")
            nc.sync.dma_start(out=dst_t[:], in_=dst_d[:])
            ew_t = sb.tile([128, NCHUNK], F32, tag="ew")
            nc.sync.dma_start(out=ew_t[:], in_=ew_d[:])
            cw_t = []
            for l in range(3):
                t = sb.tile([IN_C if l == 0 else HID, HID], BF16, tag=f"cw{l}")
                nc.sync.dma_start(out=t[:], in_=cw_d[l][:])
                cw_t.append(t)
            cb_t, bng_t, bnb_t = [], [], []
            for l in range(3):
                tb = sb.tile([HID, 1], F32, tag=f"cb{l}")
                nc.sync.dma_start(out=tb[:], in_=cb_d[l][:])
                cb_t.append(tb)
                tg = sb.tile([HID, 1], F32, tag=f"bng{l}")
                nc.sync.dma_start(out=tg[:], in_=bng_d[l][:])
                bng_t.append(tg)
                te = sb.tile([HID, 1], F32, tag=f"bnb{l}")
                nc.sync.dma_start(out=te[:], in_=bnb_d[l][:])
                bnb_t.append(te)
            mw1_t = sb.tile([HID, 2 * HID], BF16, tag="mw1")
            nc.sync.dma_start(out=mw1_t[:], in_=mw1_d[:])
            mw2_t = sb.tile([2 * HID, HID], BF16, tag="mw2")
            nc.sync.dma_start(out=mw2_t[:], in_=mw2_d[:])
            mw3_t = sb.tile([HID, NCLS], BF16, tag="mw3")
            nc.sync.dma_start(out=mw3_t[:], in_=mw3_d[:])
            mb1_t = sb.tile([2 * HID, 1], F32, tag="mb1")
            nc.sync.dma_start(out=mb1_t[:], in_=mb1_d[:])
            mb2_t = sb.tile([HID, 1], F32, tag="mb2")
            nc.sync.dma_start(out=mb2_t[:], in_=mb2_d[:])
            mb3_t = sb.tile([NCLS, 1], F32, tag="mb3")
            nc.sync.dma_start(out=mb3_t[:], in_=mb3_d[:])

            x_t = sb2.tile([IN_C, NPAD], BF16, tag="hx")
            nc.sync.dma_start(out=x_t[:], in_=x_t_d[:])

            h_cur = x_t   # feature-major current activations
            oh_dram = dram.tile([NG, 128, CPG * 128], BF16, tag="ohcache")
            for l in range(3):
                cdim = IN_C if l == 0 else HID
                # ---- projection -> node-major bf16 table ----
                table = dram2.tile([NPAD, 128], BF16, tag="table")
                KB = NPT if NPT < 25 else 25
                assert NPT % KB == 0
                PB = 8 if KB % 8 == 0 else (4 if KB % 4 == 0 else 1)
                for kb in range(0, NPT, KB):
                    stage = sb2.tile([128, KB, 128], BF16, tag="stage")
                    for k0 in range(0, KB, PB):
                        pp = ps.tile([128, PB * HID], F32, tag="proj",
                                     space="PSUM")
                        for j in range(PB):
                            k = k0 + j
                            nc.tensor.matmul(
                                out=pp[:, j * HID:(j + 1) * HID],
                                lhsT=h_cur[:, (kb + k) * 128:(kb + k + 1) * 128],
                                rhs=cw_t[l][:], start=True, stop=True)
                        nc.any.tensor_copy(
                            out=stage[:, k0:k0 + PB, 0:HID],
                            in_=pp[:].rearrange("p (j f) -> p j f", j=PB))
                    tb_view = table[:].rearrange(
                        "(g k p) f -> g p k f", p=128, k=KB)[kb // KB]
                    nc.sync.dma_start(out=tb_view, in_=stage[:])

                # ---- gather + segment-sum into f32 partials ----
                # one-hots are layer-invariant: built on layer 0, cached in
                # DRAM, reloaded on layers 1-2
                partial = dram.tile([NST_ALL, HID, 512], F32, tag="partial")
                for g in range(NG):
                    idx_t = sb2.tile([128, GS // 16], I16, tag="idxr")
                    nc.sync.dma_start(
                        out=idx_t[:],
                        in_=gidx_d[:, g * (GS // 16):(g + 1) * (GS // 16)])
                    m_t = sb.tile([128, CPG, 128], BF16, tag="msg")
                    if not SKIP_GATHER:
                        nc.gpsimd.dma_gather(
                            out_ap=m_t[:], in_ap=table[:], idxs_ap=idx_t[:],
                            num_idxs=GS, num_idxs_reg=GS, elem_size=128,
                            single_packet=False)
                    ob_t = sb.tile([128, CPG, 128], BF16, tag="obig")
                    if l > 0:
                        nc.sync.dma_start(out=ob_t[:], in_=oh_dram[g])
                    for s in range(GST):
                        st = g * GST + s
                        agg = ps.tile([HID, 512], F32, tag="agg", space="PSUM")
                        for t4 in range(4):
                            gchunk = (st * 4 + t4) * C
                            for ci in range(C):
                                k = gchunk + ci
                                kk = (s * 4 + t4) * C + ci
                                if SKIP_CHUNKS:
                                    continue
                                if l == 0:
                                    nc.any.tensor_scalar(
                                        out=ob_t[:, kk, :], in0=iota_t[:],
                                        scalar1=dst_t[:, k:k + 1],
                                        scalar2=ew_t[:, k:k + 1],
                                        op0=ALU.is_equal, op1=ALU.mult)
                                nc.tensor.matmul(
                                    out=agg[:, t4 * 128:(t4 + 1) * 128],
                                    lhsT=m_t[:, kk, 0:HID],
                                    rhs=ob_t[:, kk, :],
                                    start=(ci == 0), stop=(ci == C - 1))
                        pstage = sb2.tile([HID, 512], F32, tag="pstage")
                        if SKIP_CHUNKS:
                            nc.vector.memset(agg[:], 0.0)
                        nc.any.tensor_copy(out=pstage[:], in_=agg[:])
                        nc.sync.dma_start(out=partial[st], in_=pstage[:])
                    if l == 0:
                        nc.sync.dma_start(out=oh_dram[g], in_=ob_t[:])

                # ---- ReduceScatter: each core gets its own node range ----
                rs_out = dram.tile([NST, HID, 512], F32, tag="rsout")
                if not SKIP_CC:
                    nc.gpsimd.collective_compute(
                        "ReduceScatter", ALU.add, replica_groups=rg,
                        ins=[partial[:]], outs=[rs_out[:]])
                h_raw = sb.tile([HID, NPAD], BF16, tag="hraw")
                nc.gpsimd.dma_start(
                    out=h_raw[:].rearrange("p (s n) -> p s n", s=NST),
                    in_=rs_out[:].rearrange("s p n -> p s n"))
                if dbg and l == 0:
                    nc.sync.dma_start(out=dbg_tab[:], in_=table[:])
                    draw = sb.tile([HID, NPAD], F32, tag="draw")
                    nc.vector.tensor_copy(out=draw[:], in_=h_raw[:])
                    nc.sync.dma_start(out=dbg_raw[:], in_=draw[:])

                # ---- bias + relu (+ sum accum), stats, BN ----
                h_rel = sb.tile([HID, NPAD], BF16, tag="hrel")
                nc.vector.tensor_scalar(
                    out=h_rel[:], in0=h_raw[:], scalar1=cb_t[l][:],
                    scalar2=0.0, op0=ALU.add, op1=ALU.max)
                ssum = sb.tile([HID, 1], F32, tag="ssum")
                nc.scalar.activation(out=h_raw[:], in_=h_rel[:],
                                     func=ACTF.Copy, accum_out=ssum[:])
                ssq = sb.tile([HID, 1], F32, tag="ssq")
                nc.scalar.activation(out=h_raw[:], in_=h_rel[:],
                                     func=ACTF.Square, accum_out=ssq[:])
                stats_in = dram.tile([HID, 2], F32, tag="stin")
                stats_out = dram.tile([HID, 2], F32, tag="stout")
                # pad-row correction: pads contribute relu(bias) each
                pb = sb.tile([HID, 1], F32, tag="pb")
                nc.vector.tensor_scalar(out=pb[:], in0=cb_t[l][:],
                                        scalar1=0.0, scalar2=None,
                                        op0=ALU.max)
                pb2 = sb.tile([HID, 1], F32, tag="pb2")
                nc.vector.tensor_tensor(out=pb2[:], in0=pb[:], in1=pb[:],
                                        op=ALU.mult)
                sc = sb.tile([HID, 2], F32, tag="statsc")
                nc.vector.tensor_scalar(out=sc[:, 0:1], in0=pb[:],
                                        scalar1=-NPADDING, scalar2=None,
                                        op0=ALU.mult)
                nc.vector.tensor_scalar(out=sc[:, 1:2], in0=pb2[:],
                                        scalar1=-NPADDING, scalar2=None,
                                        op0=ALU.mult)
                nc.vector.tensor_tensor(out=sc[:, 0:1], in0=sc[:, 0:1],
                                        in1=ssum[:], op=ALU.add)
                nc.vector.tensor_tensor(out=sc[:, 1:2], in0=sc[:, 1:2],
                                        in1=ssq[:], op=ALU.add)
                nc.sync.dma_start(out=stats_in[:], in_=sc[:])
                if not SKIP_CC:
                    nc.gpsimd.collective_compute(
                        "AllReduce", ALU.add, replica_groups=rg,
                        ins=[stats_in[:]], outs=[stats_out[:]])
                st_sb = sb.tile([HID, 2], F32, tag="stsb")
                nc.sync.dma_start(out=st_sb[:], in_=stats_out[:])
                mt = sb.tile([HID, 1], F32, tag="mt")
                nc.vector.tensor_scalar(out=mt[:], in0=st_sb[:, 0:1],
                                        scalar1=INVN, scalar2=None,
                                        op0=ALU.mult)
                vt = sb.tile([HID, 1], F32, tag="vt")
                nc.vector.tensor_scalar(out=vt[:], in0=st_sb[:, 1:2],
                                        scalar1=INVN, scalar2=None,
                                        op0=ALU.mult)
                msq = sb.tile([HID, 1], F32, tag="msq")
                nc.vector.tensor_tensor(out=msq[:], in0=mt[:], in1=mt[:],
                                        op=ALU.mult)
                nc.vector.tensor_tensor(out=vt[:], in0=vt[:], in1=msq[:],
                                        op=ALU.subtract)
                nc.vector.tensor_scalar(out=vt[:], in0=vt[:], scalar1=EPS,
                                        scalar2=None, op0=ALU.add)
                sqv = sb.tile([HID, 1], F32, tag="sqv")
                nc.scalar.activation(out=sqv[:], in_=vt[:], func=ACTF.Sqrt)
                rstd = sb.tile([HID, 1], F32, tag="rstd")
                nc.vector.reciprocal(out=rstd[:], in_=sqv[:])
                s_t = sb.tile([HID, 1], F32, tag="sT")
                nc.vector.tensor_tensor(out=s_t[:], in0=bng_t[l][:],
                                        in1=rstd[:], op=ALU.mult)
                t_t = sb.tile([HID, 1], F32, tag="tT")
                nc.vector.tensor_tensor(out=t_t[:], in0=mt[:], in1=s_t[:],
                                        op=ALU.mult)
                nc.vector.tensor_tensor(out=t_t[:], in0=bnb_t[l][:],
                                        in1=t_t[:], op=ALU.subtract)
                h_new = sb2.tile([HID, NPAD], BF16, tag="hx")
                nc.vector.tensor_scalar(out=h_new[:], in0=h_rel[:],
                                        scalar1=s_t[:], scalar2=t_t[:],
                                        op0=ALU.mult, op1=ALU.add)
                if dbg and l == 0:
                    dh = sb.tile([HID, NPAD], F32, tag="dh")
                    nc.vector.tensor_copy(out=dh[:], in_=h_new[:])
                    nc.sync.dma_start(out=dbg_h[:], in_=dh[:])
                h_cur = h_new

            # ---- MLP head (feature-major) ----
            for s in range(NST):
                p1 = ps1.tile([2 * HID, 512], F32, tag="mp1", space="PSUM")
                nc.tensor.matmul(out=p1[:], lhsT=mw1_t[:],
                                 rhs=h_cur[:, s * 512:(s + 1) * 512],
                                 start=True, stop=True)
                a1 = sb2.tile([2 * HID, 512], BF16, tag="a1")
                nc.scalar.activation(out=a1[:], in_=p1[:], func=ACTF.Gelu,
                                     bias=mb1_t[:])
                p2 = ps1.tile([HID, 512], F32, tag="mp2", space="PSUM")
                nc.tensor.matmul(out=p2[:], lhsT=mw2_t[:], rhs=a1[:],
                                 start=True, stop=True)
                a2 = sb2.tile([HID, 512], BF16, tag="a2")
                nc.scalar.activation(out=a2[:], in_=p2[:], func=ACTF.Gelu,
                                     bias=mb2_t[:])
                p3 = ps1.tile([NCLS, 512], F32, tag="mp3", space="PSUM")
                nc.tensor.matmul(out=p3[:], lhsT=mw3_t[:], rhs=a2[:],
                                 start=True, stop=True)
                ob = sb2.tile([NCLS, 512], F32, tag="ob")
                nc.vector.tensor_scalar(out=ob[:], in0=p3[:],
                                        scalar1=mb3_t[:], scalar2=None,
                                        op0=ALU.add)
                nc.sync.dma_start(out=out_d[:, s * 512:(s + 1) * 512],
                                  in_=ob[:])
    nc.compile()
    return nc


def kernel(x, edge_index, edge_attr,
           conv_w0, conv_b0, conv_w1, conv_b1, conv_w2, conv_b2,
           bn_g0, bn_be0, bn_g1, bn_be1, bn_g2, bn_be2,
           mlp_w1, mlp_b1, mlp_w2, mlp_b2, mlp_w3, mlp_b3):
    x = np.asarray(x)
    N, in_c = x.shape
    hid = np.asarray(conv_w0).shape[1]
    ncls = np.asarray(mlp_w3).shape[1]
    cfg = _cfg(N, in_c, hid, ncls)

    # gather group size: GST supertiles per dma_gather
    GST = 4

    # need C before GS; compute counts first via a cheap pre-pass
    per_core, C, NCHUNK, NSLOT = None, None, None, None
    cfg["GS"] = None
    # C depends only on edge distribution
    src = np.asarray(edge_index[0], dtype=np.int64)
    dst = np.asarray(edge_index[1], dtype=np.int64)
    owner = np.minimum(src // cfg["SHARD"], NCORES - 1)
    dstp = (dst // cfg["SHARD"]) * cfg["NPAD"] + (dst % cfg["SHARD"])
    NT_ALL = cfg["NTILES"] * NCORES
    counts = np.zeros(NCORES * NT_ALL, dtype=np.int64)
    np.add.at(counts, owner * NT_ALL + dstp // 128, 1)
    C = int(max(1, -(-counts.max() // 128)))
    cfg["GS"] = GST * 4 * C * 128

    per_core, C2, NCHUNK, NSLOT = _preprocess(x, edge_index, edge_attr, cfg)
    assert C2 == C

    bf = ml_dtypes.bfloat16
    common = dict(
        iota128=np.tile(np.arange(128, dtype=np.float32).astype(bf), (128, 1)),
        conv_w0=np.asarray(conv_w0).astype(bf),
        conv_w1=np.asarray(conv_w1).astype(bf),
        conv_w2=np.asarray(conv_w2).astype(bf),
        conv_b0=np.asarray(conv_b0, dtype=np.float32).reshape(-1, 1),
        conv_b1=np.asarray(conv_b1, dtype=np.float32).reshape(-1, 1),
        conv_b2=np.asarray(conv_b2, dtype=np.float32).reshape(-1, 1),
        bn_g0=np.asarray(bn_g0, dtype=np.float32).reshape(-1, 1),
        bn_g1=np.asarray(bn_g1, dtype=np.float32).reshape(-1, 1),
        bn_g2=np.asarray(bn_g2, dtype=np.float32).reshape(-1, 1),
        bn_be0=np.asarray(bn_be0, dtype=np.float32).reshape(-1, 1),
        bn_be1=np.asarray(bn_be1, dtype=np.float32).reshape(-1, 1),
        bn_be2=np.asarray(bn_be2, dtype=np.float32).reshape(-1, 1),
        mlp_w1=np.asarray(mlp_w1).astype(bf),
        mlp_w2=np.asarray(mlp_w2).astype(bf),
        mlp_w3=np.asarray(mlp_w3).astype(bf),
        mlp_b1=np.asarray(mlp_b1, dtype=np.float32).reshape(-1, 1),
        mlp_b2=np.asarray(mlp_b2, dtype=np.float32).reshape(-1, 1),
        mlp_b3=np.asarray(mlp_b3, dtype=np.float32).reshape(-1, 1),
    )
    in_maps = []
    for c in range(NCORES):
        m = dict(common)
        m["x_t"] = per_core[c]["x_t"]
        m["g_idx"] = per_core[c]["g_idx"]
        m["dst_rel"] = per_core[c]["dst_rel"]
        m["ew_s"] = per_core[c]["ew_s"]
        in_maps.append(m)

    nc = _build(cfg, C, NCHUNK, NSLOT, GST)
    nc.m = get_hw_module(nc.m)
    trace = bool(os.environ.get("KERNEL_TRACE"))
    try:
        res = bass_utils.run_bass_kernel_spmd(
            nc, in_maps, core_ids=list(range(NCORES)), trace=trace)
    except ModuleNotFoundError:
        trace = False
        res = bass_utils.run_bass_kernel_spmd(
            nc, in_maps, core_ids=list(range(NCORES)))
    if trace and res.exec_time_ns is not None:
        print(f"HW exec time: {res.exec_time_ns} ns")
    if os.environ.get("KERNEL_TIME"):
        import time as _t
        for it in range(2):
            t0 = _t.time()
            res = bass_utils.run_bass_kernel_spmd(
                nc, in_maps, core_ids=list(range(NCORES)))
            print(f"warm run {it}: {(_t.time()-t0)*1e3:.1f} ms")

    kernel._last_res = res
    kernel._last_nc = nc
    kernel._last_in_maps = in_maps
    out = np.empty((N, cfg["NCLS"]), dtype=np.float32)
    SHARD = cfg["SHARD"]
    for c in range(NCORES):
        out[c * SHARD:(c + 1) * SHARD] = res.results[c]["out5"][:, :SHARD].T
    return out



# revision 13
# speedup vs baseline: 1.3347x; 1.3347x over previous
"""Trainium2 Bass kernel for MineralDepositGCN (3x GCNConv+BN + MLP head).

Strategy (8 NeuronCores, SPMD single program), per sharding hint:
  - Nodes sharded by range: core c owns nodes [c*12500, (c+1)*12500),
    padded to NPAD=12800 per core. Edges owned by the DST core, so the
    scatter-add is fully device-local (PSUM accumulation, no
    ReduceScatter of partials).
  - Per layer: each core projects its own shard h@W into a compact
    node-major f32 table [NPAD, 64] (256B rows); AllGather -> full table
    [8*NPAD, 64] in DRAM (the "halo" exchange, 26MB).
  - Edge messages fetched with dma_gather (elem 256B, all-useful f32).
    int16 gather indices only reach 32767 rows, so edges are bucketed by
    src-core-pair "window" (4 windows x 25600 rows) and gathered from a
    sliced table view.
  - Scatter-add via one-hot matmuls: edges grouped into 128-slot chunks
    per (128-dst-tile, window) cell; one-hot built on-device each layer
    with a broadcast is_equal against an iota row (DVE), messages
    pre-scaled by edge weight (DVE, broadcast mult); TensorE accumulates
    chunks into a [64, 512] PSUM tile per 512-dst supertile.
  - Bias+ReLU fused on PSUM drain; BN stats via 2 accum passes + tiny
    AllReduce [64,2] with closed-form pad-row correction; MLP head
    feature-major per supertile.
"""
import os
import numpy as np
import ml_dtypes

from concourse import bass, bacc, tile, mybir
from concourse import bass_utils
from concourse.bass_interp import get_hw_module

BF16 = mybir.dt.bfloat16
F32 = mybir.dt.float32
I16 = mybir.dt.int16
ALU = mybir.AluOpType
ACTF = mybir.ActivationFunctionType

NCORES = 8
EPS = 1e-5


def _cfg(n_nodes, in_c, hid, ncls):
    shard = n_nodes // NCORES
    npad = ((shard + 511) // 512) * 512
    return dict(
        N=n_nodes, IN_C=in_c, HID=hid, NCLS=ncls,
        SHARD=shard, NPAD=npad,
        NTILES=npad // 128,
        NST=npad // 512,
        NTOT=npad * NCORES,
        WINR=2 * npad,            # window rows (2 src cores per window)
        NWIN=NCORES // 2,
    )


def _plan(edge_index, cfg):
    """Shared (all-core) chunk structure: per-cell chunk counts + call list.

    Cells ordered (st, w, t4); cell = ((st*NWIN)+w)*4 + t4. Returns
    C[ncell], chunk_base[ncell], NCHUNK, calls[(st,w)] = (cb0, cb1).
    """
    SHARD, NPAD, NWIN = cfg["SHARD"], cfg["NPAD"], cfg["NWIN"]
    NST = cfg["NST"]
    src = edge_index[0].astype(np.int64)
    dst = edge_index[1].astype(np.int64)
    d_owner = np.minimum(dst // SHARD, NCORES - 1)
    d_local = dst - d_owner * SHARD
    tile_g = d_local // 128
    st = tile_g // 4
    t4 = tile_g % 4
    s_owner = np.minimum(src // SHARD, NCORES - 1)
    w = s_owner // 2
    ncell = NST * NWIN * 4
    cell = (st * NWIN + w) * 4 + t4
    counts = np.zeros((NCORES, ncell), dtype=np.int64)
    np.add.at(counts.reshape(-1), d_owner * ncell + cell, 1)
    cmax = counts.max(axis=0)
    C = -(-cmax // 128)                      # ceil
    # every (st, t4) needs >=1 chunk so PSUM gets cleared (pad tiles)
    C4 = C.reshape(NST, NWIN, 4)
    for s in range(NST):
        for t in range(4):
            if C4[s, :, t].sum() == 0:
                C4[s, 0, t] = 1
    C = C4.reshape(-1)
    chunk_base = np.zeros(ncell + 1, dtype=np.int64)
    chunk_base[1:] = np.cumsum(C)
    nchunk = int(chunk_base[-1])
    calls = {}
    for s in range(NST):
        for wi in range(NWIN):
            c0 = (s * NWIN + wi) * 4
            cb0 = int(chunk_base[c0])
            cb1 = int(chunk_base[c0 + 4])
            calls[(s, wi)] = (cb0, cb1)
    return C, chunk_base, nchunk, calls


def _preprocess(x, edge_index, edge_attr, cfg, C, chunk_base, nchunk):
    """Per-core inputs: x shard + slotted edge data in the shared layout."""
    SHARD, NPAD, NWIN = cfg["SHARD"], cfg["NPAD"], cfg["NWIN"]
    NST = cfg["NST"]
    WINR = cfg["WINR"]
    NSLOT = nchunk * 128
    src = edge_index[0].astype(np.int64)
    dst = edge_index[1].astype(np.int64)
    ew = np.asarray(edge_attr, dtype=np.float32)

    d_owner = np.minimum(dst // SHARD, NCORES - 1)
    d_local = dst - d_owner * SHARD
    tile_g = d_local // 128
    dst_rel = d_local % 128
    s_owner = np.minimum(src // SHARD, NCORES - 1)
    s_local = src - s_owner * SHARD
    s_pad = s_owner * NPAD + s_local
    w = s_owner // 2
    idx_loc = s_pad - w * WINR
    cell = ((tile_g // 4) * NWIN + w) * 4 + (tile_g % 4)

    bf = ml_dtypes.bfloat16
    per_core = []
    for c in range(NCORES):
        m = d_owner == c
        ce = cell[m]
        il = idx_loc[m]
        dr = dst_rel[m]
        we = ew[m]
        sp = s_pad[m]
        order = np.lexsort((sp, ce))         # by cell, then src (locality)
        ce, il, dr, we = ce[order], il[order], dr[order], we[order]
        cnt = np.bincount(ce, minlength=len(C))
        starts = np.zeros(len(C), dtype=np.int64)
        starts[1:] = np.cumsum(cnt)[:-1]
        rank = np.arange(ce.shape[0], dtype=np.int64) - starts[ce]
        slot = chunk_base[ce] * 128 + rank

        idx16 = np.zeros(NSLOT, dtype=np.int16)
        dstb = np.full(NSLOT, 255.0, dtype=np.float32)
        ewb = np.zeros(NSLOT, dtype=np.float32)
        idx16[slot] = il.astype(np.int16)
        dstb[slot] = dr.astype(np.float32)
        ewb[slot] = we

        # wrap indices: slot i of each call block at [i%16, blockcol + i//16]
        g_idx = idx16.reshape(NSLOT // 16, 16).T.copy()   # [16, NSLOT/16]

        per_core.append(dict(
            g_idx=g_idx,
            dst_rel=dstb.reshape(nchunk, 128).T.astype(bf),
            ew_s=ewb.reshape(nchunk, 128).T.astype(bf),
        ))

    for c in range(NCORES):
        xs = np.zeros((cfg["IN_C"], NPAD), dtype=np.float32)
        xs[:, :SHARD] = np.asarray(x[c * SHARD:(c + 1) * SHARD]).T
        per_core[c]["x_t"] = xs.astype(bf)
    return per_core, NSLOT


def _build(cfg, C, chunk_base, nchunk, calls):
    IN_C, HID, NCLS = cfg["IN_C"], cfg["HID"], cfg["NCLS"]
    NPAD, NTILES, NST = cfg["NPAD"], cfg["NTILES"], cfg["NST"]
    NTOT, WINR, NWIN = cfg["NTOT"], cfg["WINR"], cfg["NWIN"]
    NSLOT = nchunk * 128
    NPADDING = float(NCORES * NPAD - cfg["N"])
    INVN = 1.0 / cfg["N"]
    C4 = C.reshape(NST, NWIN, 4)
    CPGMAX = max(cb1 - cb0 for (cb0, cb1) in calls.values())
    CMAX = int(C.max())
    LVL = int(os.environ.get("KLEVEL", "0"))

    nc = bacc.Bacc("TRN2", target_bir_lowering=False, debug=False,
                   num_devices=NCORES)

    def din(name, shape, dt):
        return nc.dram_tensor(name, shape, dt, kind="ExternalInput").ap()

    x_t_d = din("x_t", [IN_C, NPAD], BF16)
    gidx_d = din("g_idx", [16, NSLOT // 16], I16)
    dst_d = din("dst_rel", [128, nchunk], BF16)
    ew_d = din("ew_s", [128, nchunk], BF16)
    iota_d = din("iota128", [128, 128], BF16)
    cw_d = [din(f"conv_w{l}", [IN_C if l == 0 else HID, HID], BF16)
            for l in range(3)]
    cb_d = [din(f"conv_b{l}", [HID, 1], F32) for l in range(3)]
    bng_d = [din(f"bn_g{l}", [HID, 1], F32) for l in range(3)]
    bnb_d = [din(f"bn_be{l}", [HID, 1], F32) for l in range(3)]
    mw1_d = din("mlp_w1", [HID, 2 * HID], BF16)
    mw2_d = din("mlp_w2", [2 * HID, HID], BF16)
    mw3_d = din("mlp_w3", [HID, NCLS], BF16)
    mb1_d = din("mlp_b1", [2 * HID, 1], F32)
    mb2_d = din("mlp_b2", [HID, 1], F32)
    mb3_d = din("mlp_b3", [NCLS, 1], F32)
    out_d = nc.dram_tensor("out5", [NCLS, NPAD], F32,
                           kind="ExternalOutput").ap()
    dbg = bool(os.environ.get("KDBG"))
    if dbg:
        dbg_tab = nc.dram_tensor("dbg_tab", [NTOT, 64], F32,
                                 kind="ExternalOutput").ap()
        dbg_hrel = nc.dram_tensor("dbg_hrel", [HID, NPAD], F32,
                                  kind="ExternalOutput").ap()
        dbg_h = nc.dram_tensor("dbg_h", [HID, NPAD], F32,
                               kind="ExternalOutput").ap()

    rg = [list(range(NCORES))]

    with tile.TileContext(nc) as tc:
        with tc.tile_pool(name="sb", bufs=1) as sb, \
             tc.tile_pool(name="sb2", bufs=2) as sb2, \
             tc.tile_pool(name="sbA", bufs=1) as sbA, \
             tc.tile_pool(name="mp", bufs=2) as mp, \
             tc.tile_pool(name="msp", bufs=5) as msp, \
             tc.tile_pool(name="ohp", bufs=3) as ohp, \
             tc.tile_pool(name="idxp", bufs=2) as idxp, \
             tc.tile_pool(name="dbgp", bufs=1) as dbgp, \
             tc.tile_pool(name="psP", bufs=2, space="PSUM") as psP, \
             tc.tile_pool(name="psA", bufs=3, space="PSUM") as psA, \
             tc.tile_pool(name="psM", bufs=1, space="PSUM") as psM, \
             tc.tile_pool(name="dram", bufs=1, space="DRAM") as dram, \
             tc.tile_pool(name="dram2", bufs=2, space="DRAM") as dram2:

            # ---- persistent loads ----
            iota_t = sb.tile([128, 128], BF16, tag="iota")
            nc.sync.dma_start(out=iota_t[:], in_=iota_d[:])
            dst_t = sb.tile([128, nchunk], BF16, tag="dst")
            nc.sync.dma_start(out=dst_t[:], in_=dst_d[:])
            ew_f = sb.tile([128, nchunk], F32, tag="ewf")
            EWCH = 512
            for e0 in range(0, nchunk, EWCH):
                e1 = min(e0 + EWCH, nchunk)
                ewt = sb2.tile([128, EWCH], BF16, tag="ewtmp")
                nc.sync.dma_start(out=ewt[:, 0:e1 - e0], in_=ew_d[:, e0:e1])
                nc.vector.tensor_copy(out=ew_f[:, e0:e1],
                                      in_=ewt[:, 0:e1 - e0])
            cw_t = []
            for l in range(3):
                t = sb.tile([IN_C if l == 0 else HID, HID], BF16, tag=f"cw{l}")
                nc.sync.dma_start(out=t[:], in_=cw_d[l][:])
                cw_t.append(t)
            cb_t, bng_t, bnb_t = [], [], []
            for l in range(3):
                tb = sb.tile([HID, 1], F32, tag=f"cb{l}")
                nc.sync.dma_start(out=tb[:], in_=cb_d[l][:])
                cb_t.append(tb)
                tg = sb.tile([HID, 1], F32, tag=f"bng{l}")
                nc.sync.dma_start(out=tg[:], in_=bng_d[l][:])
                bng_t.append(tg)
                te = sb.tile([HID, 1], F32, tag=f"bnb{l}")
                nc.sync.dma_start(out=te[:], in_=bnb_d[l][:])
                bnb_t.append(te)
            mw1_t = sb.tile([HID, 2 * HID], BF16, tag="mw1")
            nc.sync.dma_start(out=mw1_t[:], in_=mw1_d[:])
            mw2_t = sb.tile([2 * HID, HID], BF16, tag="mw2")
            nc.sync.dma_start(out=mw2_t[:], in_=mw2_d[:])
            mw3_t = sb.tile([HID, NCLS], BF16, tag="mw3")
            nc.sync.dma_start(out=mw3_t[:], in_=mw3_d[:])
            mb1_t = sb.tile([2 * HID, 1], F32, tag="mb1")
            nc.sync.dma_start(out=mb1_t[:], in_=mb1_d[:])
            mb2_t = sb.tile([HID, 1], F32, tag="mb2")
            nc.sync.dma_start(out=mb2_t[:], in_=mb2_d[:])
            mb3_t = sb.tile([NCLS, 1], F32, tag="mb3")
            nc.sync.dma_start(out=mb3_t[:], in_=mb3_d[:])

            x_t = sb2.tile([IN_C, NPAD], BF16, tag="hx")
            nc.sync.dma_start(out=x_t[:], in_=x_t_d[:])

            # replicate gather indices [16, X] -> [128, X] in DRAM
            idx_rep = dram.tile([128, NSLOT // 16], I16, tag="idxrep")
            if LVL < 3:
                for r in range(8):
                    nc.sync.dma_start(out=idx_rep[16 * r:16 * (r + 1), :],
                                      in_=gidx_d[:])

            h_cur = x_t
            for l in range(3):
                cdim = IN_C if l == 0 else HID
                # ---- projection -> compact node-major f32 table ----
                tab_in = dram2.tile([NPAD, 64], F32, tag="tabin")
                PB = 4
                for g in range(NTILES // PB):
                    pp = psP.tile([128, PB * HID], F32, tag="proj",
                                  space="PSUM")
                    for j in range(PB):
                        k = g * PB + j
                        nc.tensor.matmul(
                            out=pp[:, j * HID:(j + 1) * HID],
                            lhsT=h_cur[0:cdim, k * 128:(k + 1) * 128],
                            rhs=cw_t[l][:], start=True, stop=True)
                    stg = sb2.tile([128, PB, HID], F32, tag="stage")
                    nc.any.tensor_copy(
                        out=stg[:],
                        in_=pp[:].rearrange("p (j f) -> p j f", j=PB))
                    tb_view = tab_in[:].rearrange(
                        "(g j p) f -> g p j f", p=128, j=PB)[g]
                    nc.sync.dma_start(out=tb_view, in_=stg[:])

                # ---- AllGather the projected table ----
                tab_full = dram2.tile([NTOT, 64], F32, tag="tabfull")
                if LVL < 4:
                    nc.gpsimd.collective_compute(
                        "AllGather", ALU.bypass, replica_groups=rg,
                        ins=[tab_in[:]], outs=[tab_full[:]])

                # ---- gather + one-hot scatter matmuls, local dst ----
                h_rel = sbA.tile([HID, NPAD], BF16, tag="hrel")
                for s in range(NST):
                    agg = psA.tile([HID, 512], F32, tag="agg", space="PSUM")
                    if LVL >= 1:
                        nc.vector.memset(agg[:], 0.0)
                    ms_w = {}
                    for wi in range(NWIN):
                        cb0, cb1 = calls[(s, wi)]
                        cpg = cb1 - cb0
                        if cpg == 0:
                            continue
                        gs = cpg * 128
                        if LVL < 3:
                            idx_t = idxp.tile([128, CPGMAX * 8], I16,
                                              tag="idx")
                            nc.sync.dma_start(
                                out=idx_t[:, 0:gs // 16],
                                in_=idx_rep[:, cb0 * 8:cb1 * 8])
                            m_t = mp.tile([128, CPGMAX, 64], F32, tag="mt")
                            nc.gpsimd.dma_gather(
                                out_ap=m_t[:, 0:cpg, :],
                                in_ap=tab_full[wi * WINR:(wi + 1) * WINR, :],
                                idxs_ap=idx_t[:, 0:gs // 16],
                                num_idxs=gs, num_idxs_reg=gs,
                                elem_size=64, single_packet=False)
                        if LVL < 2:
                            ms = msp.tile([128, CPGMAX, 64], BF16, tag="ms")
                            a0, a1 = bass.broadcast_tensor_aps(
                                m_t[:, 0:cpg, :],
                                ew_f[:, cb0:cb1].rearrange(
                                    "p (k a) -> p k a", a=1))
                            nc.vector.tensor_tensor(
                                out=ms[:, 0:cpg, :], in0=a0, in1=a1,
                                op=ALU.mult)
                            ms_w[wi] = ms
                    if LVL < 1:
                        # region-major so each PSUM accumulation group is a
                        # consecutive run of matmuls; one-hots built per cell
                        for t in range(4):
                            nseq = int(C4[s, :, t].sum())
                            done = 0
                            for wi in range(NWIN):
                                cc = int(C4[s, wi, t])
                                if cc == 0:
                                    continue
                                cb0, _ = calls[(s, wi)]
                                kk = int(C4[s, wi, 0:t].sum())
                                k0 = cb0 + kk
                                oh = ohp.tile([128, CMAX, 128], BF16,
                                              tag="oh")
                                b0, b1 = bass.broadcast_tensor_aps(
                                    iota_t[:].rearrange(
                                        "p (a f) -> p a f", a=1),
                                    dst_t[:, k0:k0 + cc].rearrange(
                                        "p (k a) -> p k a", a=1))
                                nc.vector.tensor_tensor(
                                    out=oh[:, 0:cc, :], in0=b0, in1=b1,
                                    op=ALU.is_equal)
                                for ci in range(cc):
                                    nc.tensor.matmul(
                                        out=agg[:, t * 128:(t + 1) * 128],
                                        lhsT=ms_w[wi][:, kk + ci, :],
                                        rhs=oh[:, ci, :],
                                        start=(done == 0),
                                        stop=(done == nseq - 1))
                                    done += 1
                    # drain + bias + relu
                    nc.vector.tensor_scalar(
                        out=h_rel[:, s * 512:(s + 1) * 512], in0=agg[:],
                        scalar1=cb_t[l][:], scalar2=0.0,
                        op0=ALU.add, op1=ALU.max)

                if dbg and l == 0:
                    nc.sync.dma_start(out=dbg_tab[:], in_=tab_full[:])
                    dh0 = dbgp.tile([HID, NPAD], F32, tag="dh0")
                    nc.vector.tensor_copy(out=dh0[:], in_=h_rel[:])
                    nc.sync.dma_start(out=dbg_hrel[:], in_=dh0[:])

                # ---- BN stats (global, with pad-row correction) ----
                h_new = sb2.tile([IN_C, NPAD], BF16, tag="hx")
                scr = h_new[0:HID, :]
                ssum = sb.tile([HID, 1], F32, tag="ssum")
                nc.scalar.activation(out=scr, in_=h_rel[:],
                                     func=ACTF.Copy, accum_out=ssum[:])
                ssq = sb.tile([HID, 1], F32, tag="ssq")
                nc.scalar.activation(out=scr, in_=h_rel[:],
                                     func=ACTF.Square, accum_out=ssq[:])
                pb = sb.tile([HID, 1], F32, tag="pb")
                nc.vector.tensor_scalar(out=pb[:], in0=cb_t[l][:],
                                        scalar1=0.0, scalar2=None,
                                        op0=ALU.max)
                pb2 = sb.tile([HID, 1], F32, tag="pb2")
                nc.vector.tensor_tensor(out=pb2[:], in0=pb[:], in1=pb[:],
                                        op=ALU.mult)
                sc = sb.tile([HID, 2], F32, tag="statsc")
                nc.vector.tensor_scalar(out=sc[:, 0:1], in0=pb[:],
                                        scalar1=-NPADDING, scalar2=None,
                                        op0=ALU.mult)
                nc.vector.tensor_scalar(out=sc[:, 1:2], in0=pb2[:],
                                        scalar1=-NPADDING, scalar2=None,
                                        op0=ALU.mult)
                nc.vector.tensor_tensor(out=sc[:, 0:1], in0=sc[:, 0:1],
                                        in1=ssum[:], op=ALU.add)
                nc.vector.tensor_tensor(out=sc[:, 1:2], in0=sc[:, 1:2],
                                        in1=ssq[:], op=ALU.add)
                stats_in = dram.tile([HID, 2], F32, tag=f"stin{l}")
                stats_out = dram.tile([HID, 2], F32, tag=f"stout{l}")
                st_sb = sb.tile([HID, 2], F32, tag="stsb")
                if LVL < 4:
                    nc.sync.dma_start(out=stats_in[:], in_=sc[:])
                    nc.gpsimd.collective_compute(
                        "AllReduce", ALU.add, replica_groups=rg,
                        ins=[stats_in[:]], outs=[stats_out[:]])
                    nc.sync.dma_start(out=st_sb[:], in_=stats_out[:])
                else:
                    nc.vector.tensor_copy(out=st_sb[:], in_=sc[:])
                mt = sb.tile([HID, 1], F32, tag="mt1")
                nc.vector.tensor_scalar(out=mt[:], in0=st_sb[:, 0:1],
                                        scalar1=INVN, scalar2=None,
                                        op0=ALU.mult)
                vt = sb.tile([HID, 1], F32, tag="vt")
                nc.vector.tensor_scalar(out=vt[:], in0=st_sb[:, 1:2],
                                        scalar1=INVN, scalar2=None,
                                        op0=ALU.mult)
                msq = sb.tile([HID, 1], F32, tag="msq")
                nc.vector.tensor_tensor(out=msq[:], in0=mt[:], in1=mt[:],
                                        op=ALU.mult)
                nc.vector.tensor_tensor(out=vt[:], in0=vt[:], in1=msq[:],
                                        op=ALU.subtract)
                nc.vector.tensor_scalar(out=vt[:], in0=vt[:], scalar1=EPS,
                                        scalar2=None, op0=ALU.add)
                sqv = sb.tile([HID, 1], F32, tag="sqv")
                nc.scalar.activation(out=sqv[:], in_=vt[:], func=ACTF.Sqrt)
                rstd = sb.tile([HID, 1], F32, tag="rstd")
                nc.vector.reciprocal(out=rstd[:], in_=sqv[:])
                s_t = sb.tile([HID, 1], F32, tag="sT")
                nc.vector.tensor_tensor(out=s_t[:], in0=bng_t[l][:],
                                        in1=rstd[:], op=ALU.mult)
                t_t = sb.tile([HID, 1], F32, tag="tT")
                nc.vector.tensor_tensor(out=t_t[:], in0=mt[:], in1=s_t[:],
                                        op=ALU.mult)
                nc.vector.tensor_tensor(out=t_t[:], in0=bnb_t[l][:],
                                        in1=t_t[:], op=ALU.subtract)
                nc.vector.tensor_scalar(out=h_new[0:HID, :], in0=h_rel[:],
                                        scalar1=s_t[:], scalar2=t_t[:],
                                        op0=ALU.mult, op1=ALU.add)
                if dbg and l == 0:
                    dh1 = dbgp.tile([HID, NPAD], F32, tag="dh1")
                    nc.vector.tensor_copy(out=dh1[:], in_=h_new[0:HID, :])
                    nc.sync.dma_start(out=dbg_h[:], in_=dh1[:])
                h_cur = h_new

            # ---- MLP head (feature-major) ----
            for s in range(NST):
                p1 = psM.tile([2 * HID, 512], F32, tag="mp1", space="PSUM")
                nc.tensor.matmul(out=p1[:], lhsT=mw1_t[:],
                                 rhs=h_cur[0:HID, s * 512:(s + 1) * 512],
                                 start=True, stop=True)
                a1 = sb2.tile([2 * HID, 512], BF16, tag="a1")
                nc.scalar.activation(out=a1[:], in_=p1[:], func=ACTF.Gelu,
                                     bias=mb1_t[:])
                p2 = psM.tile([HID, 512], F32, tag="mp2", space="PSUM")
                nc.tensor.matmul(out=p2[:], lhsT=mw2_t[:], rhs=a1[:],
                                 start=True, stop=True)
                a2 = sb2.tile([HID, 512], BF16, tag="a2")
                nc.scalar.activation(out=a2[:], in_=p2[:], func=ACTF.Gelu,
                                     bias=mb2_t[:])
                p3 = psM.tile([NCLS, 512], F32, tag="mp3", space="PSUM")
                nc.tensor.matmul(out=p3[:], lhsT=mw3_t[:], rhs=a2[:],
                                 start=True, stop=True)
                ob = sb2.tile([NCLS, 512], F32, tag="ob")
                nc.vector.tensor_scalar(out=ob[:], in0=p3[:],
                                        scalar1=mb3_t[:], scalar2=None,
                                        op0=ALU.add)
                nc.sync.dma_start(out=out_d[:, s * 512:(s + 1) * 512],
                                  in_=ob[:])
    nc.compile()
    return nc


def kernel(x, edge_index, edge_attr,
           conv_w0, conv_b0, conv_w1, conv_b1, conv_w2, conv_b2,
           bn_g0, bn_be0, bn_g1, bn_be1, bn_g2, bn_be2,
           mlp_w1, mlp_b1, mlp_w2, mlp_b2, mlp_w3, mlp_b3):
    x = np.asarray(x)
    N, in_c = x.shape
    hid = np.asarray(conv_w0).shape[1]
    ncls = np.asarray(mlp_w3).shape[1]
    cfg = _cfg(N, in_c, hid, ncls)

    edge_index = np.asarray(edge_index)
    C, chunk_base, nchunk, calls = _plan(edge_index, cfg)
    per_core, NSLOT = _preprocess(x, edge_index, edge_attr, cfg,
                                  C, chunk_base, nchunk)

    bf = ml_dtypes.bfloat16
    common = dict(
        iota128=np.tile(np.arange(128, dtype=np.float32).astype(bf),
                        (128, 1)),
        conv_w0=np.asarray(conv_w0).astype(bf),
        conv_w1=np.asarray(conv_w1).astype(bf),
        conv_w2=np.asarray(conv_w2).astype(bf),
        conv_b0=np.asarray(conv_b0, dtype=np.float32).reshape(-1, 1),
        conv_b1=np.asarray(conv_b1, dtype=np.float32).reshape(-1, 1),
        conv_b2=np.asarray(conv_b2, dtype=np.float32).reshape(-1, 1),
        bn_g0=np.asarray(bn_g0, dtype=np.float32).reshape(-1, 1),
        bn_g1=np.asarray(bn_g1, dtype=np.float32).reshape(-1, 1),
        bn_g2=np.asarray(bn_g2, dtype=np.float32).reshape(-1, 1),
        bn_be0=np.asarray(bn_be0, dtype=np.float32).reshape(-1, 1),
        bn_be1=np.asarray(bn_be1, dtype=np.float32).reshape(-1, 1),
        bn_be2=np.asarray(bn_be2, dtype=np.float32).reshape(-1, 1),
        mlp_w1=np.asarray(mlp_w1).astype(bf),
        mlp_w2=np.asarray(mlp_w2).astype(bf),
        mlp_w3=np.asarray(mlp_w3).astype(bf),
        mlp_b1=np.asarray(mlp_b1, dtype=np.float32).reshape(-1, 1),
        mlp_b2=np.asarray(mlp_b2, dtype=np.float32).reshape(-1, 1),
        mlp_b3=np.asarray(mlp_b3, dtype=np.float32).reshape(-1, 1),
    )
    in_maps = []
    for c in range(NCORES):
        m = dict(common)
        m["x_t"] = per_core[c]["x_t"]
        m["g_idx"] = per_core[c]["g_idx"]
        m["dst_rel"] = per_core[c]["dst_rel"]
        m["ew_s"] = per_core[c]["ew_s"]
        in_maps.append(m)

    nc = _build(cfg, C, chunk_base, nchunk, calls)
    nc.m = get_hw_module(nc.m)
    res = bass_utils.run_bass_kernel_spmd(
        nc, in_maps, core_ids=list(range(NCORES)))

    kernel._last_res = res
    kernel._last_nc = nc
    kernel._last_in_maps = in_maps
    out = np.empty((N, cfg["NCLS"]), dtype=np.float32)
    SHARD = cfg["SHARD"]
    for c in range(NCORES):
        out[c * SHARD:(c + 1) * SHARD] = res.results[c]["out5"][:, :SHARD].T
    return out


# revision 18
# speedup vs baseline: 9.4121x; 7.0521x over previous
"""Trainium2 Bass kernel for MineralDepositGCN (3x GCNConv+BN + MLP head).

Strategy (8 NeuronCores, SPMD single program), per the sharding hint:
  - Nodes sharded by range: core c owns nodes [c*12500, (c+1)*12500),
    padded to NPAD=12800. Edges owned by their DST core, so aggregation is
    device-local; the halo exchange is an AllGather of projected features.
  - Per layer: project own shard h@W into a compact node-major f32 table
    [NPAD, 64] (256B rows), AllGather -> [8*NPAD, 64] in DRAM.
  - Messages fetched with dma_gather (256B rows, all-useful f32). int16
    gather indices reach 32767 rows only, so edges are bucketed by
    src-core-pair "window" (4 windows x 25600 rows), gathered from a
    sliced table view.
  - Scatter-add via dma_scatter_add (SDMA CCE f32 accumulate) into a
    DRAM aggregate [NPAD, 64]. CCE loses concurrent duplicate updates
    within one call, so edges are split into conflict-free runs: run
    (w, k) holds each dst's k-th edge from window w; runs execute as
    separate (serialized) scatter calls. Gathers batch several runs per
    call; edge-weight scaling is one in-place broadcast multiply per
    segment. This keeps the whole edge stage at ~260 instructions/layer
    (the backend executes ~12k instructions/s, so instruction count
    dominates the runtime).
  - Aggregate drain: f32 node-major -> bf16 [6400, 128] (two nodes per
    row) -> hardware xbar dma transpose -> parity-interleaved
    feature-major h [128=(feat,parity), 6400]. Projection/BN/MLP all
    operate on this layout at unchanged instruction counts.
  - BN stats via 2 accum passes + parity fold + tiny AllReduce with
    closed-form pad-row correction.
"""
import os
import numpy as np
import ml_dtypes

from concourse import bass, bacc, tile, mybir
from concourse import bass_utils
from concourse.bass_interp import get_hw_module

BF16 = mybir.dt.bfloat16
F32 = mybir.dt.float32
I16 = mybir.dt.int16
ALU = mybir.AluOpType
ACTF = mybir.ActivationFunctionType

NCORES = 8
EPS = 1e-5
SEGMAX = 13056          # max gather-segment slots (128-mult)
SCATMAX = 4096          # max slots per dma_scatter_add call


def _cfg(n_nodes, in_c, hid, ncls):
    shard = n_nodes // NCORES
    npad = ((shard + 511) // 512) * 512
    return dict(
        N=n_nodes, IN_C=in_c, HID=hid, NCLS=ncls,
        SHARD=shard, NPAD=npad,
        NTILES=npad // 128,
        NTOT=npad * NCORES,
        WINR=2 * npad,
        NWIN=NCORES // 2,
    )


def _plan(edge_index, cfg):
    """Conflict-free run structure shared by all cores.

    Edges keyed by (window w, rank k) where k = occurrence index of the
    edge's dst within window w on its owner core. Run (w, k) has
    RL[w][k] = 128*ceil(max_core count/128) slots. Runs are packed in
    (w, k) order into gather segments of <= SEGMAX slots.
    """
    SHARD, NWIN, NPAD = cfg["SHARD"], cfg["NWIN"], cfg["NPAD"]
    src = edge_index[0].astype(np.int64)
    dst = edge_index[1].astype(np.int64)
    d_owner = np.minimum(dst // SHARD, NCORES - 1)
    d_local = dst - d_owner * SHARD
    s_owner = np.minimum(src // SHARD, NCORES - 1)
    w = s_owner // 2

    # rank of each edge within its (owner, w, dst) group
    key = (d_owner * NWIN + w) * SHARD + d_local
    order = np.argsort(key, kind="stable")
    sk = key[order]
    new = np.ones(len(sk), dtype=bool)
    new[1:] = sk[1:] != sk[:-1]
    idxs = np.arange(len(sk))
    starts = idxs[new]
    grp_start = starts[np.cumsum(new) - 1]
    rank_sorted = idxs - grp_start
    rank = np.empty(len(sk), dtype=np.int64)
    rank[order] = rank_sorted

    KMAX = int(rank.max()) + 1
    cnt = np.zeros((NCORES, NWIN, KMAX), dtype=np.int64)
    np.add.at(cnt.reshape(-1), (d_owner * NWIN + w) * KMAX + rank, 1)
    cmax = cnt.max(axis=0)                       # [NWIN, KMAX]
    RL = ((cmax + 127) // 128) * 128
    run_off = np.zeros((NWIN, KMAX), dtype=np.int64)
    segments = []                                # per window
    off = 0
    for wi in range(NWIN):
        segs = []
        cur_off, cur_n, cur_runs = off, 0, []
        for k in range(KMAX):
            rl = int(RL[wi, k])
            if rl == 0:
                continue
            if cur_n + rl > SEGMAX and cur_n > 0:
                segs.append((cur_off, cur_n, cur_runs))
                cur_off, cur_n, cur_runs = off, 0, []
            run_off[wi, k] = off
            cur_runs.append(k)
            cur_n += rl
            off += rl
        if cur_n > 0:
            segs.append((cur_off, cur_n, cur_runs))
        segments.append(segs)
    NSLOT = int(off)
    return dict(rank=rank, w=w, d_owner=d_owner, d_local=d_local,
                RL=RL, run_off=run_off, segments=segments,
                NSLOT=NSLOT, KMAX=KMAX)


def _preprocess(x, edge_index, edge_attr, cfg, plan):
    SHARD, NPAD, NWIN = cfg["SHARD"], cfg["NPAD"], cfg["NWIN"]
    WINR = cfg["WINR"]
    NSLOT = plan["NSLOT"]
    src = edge_index[0].astype(np.int64)
    ew = np.asarray(edge_attr, dtype=np.float32)
    s_owner = np.minimum(src // SHARD, NCORES - 1)
    s_local = src - s_owner * SHARD
    gidx_all = (s_owner * NPAD + s_local) - plan["w"] * WINR
    JUNK = NPAD                                  # scatter junk row

    bf = ml_dtypes.bfloat16
    per_core = []
    for c in range(NCORES):
        m = plan["d_owner"] == c
        wi = plan["w"][m]
        k = plan["rank"][m]
        gi = gidx_all[m]
        dl = plan["d_local"][m]
        we = ew[m]
        # position within run: order by (w, k, dst)
        order = np.lexsort((dl, k, wi))
        wi, k, gi, dl, we = (wi[order], k[order], gi[order], dl[order],
                             we[order])
        runkey = wi * plan["KMAX"] + k
        new = np.ones(len(runkey), dtype=bool)
        new[1:] = runkey[1:] != runkey[:-1]
        idxs = np.arange(len(runkey))
        starts = idxs[new]
        pos = idxs - starts[np.cumsum(new) - 1]
        slot = plan["run_off"][wi, k] + pos

        gidx16 = np.zeros(NSLOT, dtype=np.int16)
        sidx16 = np.full(NSLOT, JUNK, dtype=np.int16)
        ewb = np.zeros(NSLOT, dtype=np.float32)
        gidx16[slot] = gi.astype(np.int16)
        sidx16[slot] = dl.astype(np.int16)
        ewb[slot] = we

        per_core.append(dict(
            g_idx=gidx16.reshape(NSLOT // 16, 16).T.copy(),
            s_idx=sidx16.reshape(NSLOT // 16, 16).T.copy(),
            ew_s=ewb.reshape(NSLOT // 128, 128).T.astype(bf),
        ))

    for c in range(NCORES):
        xs = np.zeros((cfg["IN_C"], NPAD), dtype=np.float32)
        xs[:, :SHARD] = np.asarray(x[c * SHARD:(c + 1) * SHARD]).T
        per_core[c]["x_t"] = xs.astype(bf)
    return per_core


def _build(cfg, plan):
    IN_C, HID, NCLS = cfg["IN_C"], cfg["HID"], cfg["NCLS"]
    NPAD, NTILES = cfg["NPAD"], cfg["NTILES"]
    NTOT, WINR, NWIN = cfg["NTOT"], cfg["WINR"], cfg["NWIN"]
    NSLOT = plan["NSLOT"]
    RL, run_off, segments = plan["RL"], plan["run_off"], plan["segments"]
    NPAIR = NPAD // 2
    NPADDING = float(NCORES * NPAD - cfg["N"])
    INVN = 1.0 / cfg["N"]
    SEGC = SEGMAX // 128
    LVL = int(os.environ.get("KLEVEL", "0"))
    NCH = NSLOT // 128

    nc = bacc.Bacc("TRN2", target_bir_lowering=False, debug=False,
                   num_devices=NCORES)

    def din(name, shape, dt):
        return nc.dram_tensor(name, shape, dt, kind="ExternalInput").ap()

    x_t_d = din("x_t", [IN_C, NPAD], BF16)
    gidx_d = din("g_idx", [16, NSLOT // 16], I16)
    sidx_d = din("s_idx", [16, NSLOT // 16], I16)
    ew_d = din("ew_s", [128, NCH], BF16)
    cw_d = [din(f"conv_w{l}", [IN_C if l == 0 else HID, HID], BF16)
            for l in range(3)]
    cb_d = [din(f"conv_b{l}", [HID, 1], F32) for l in range(3)]
    bng_d = [din(f"bn_g{l}", [HID, 1], F32) for l in range(3)]
    bnb_d = [din(f"bn_be{l}", [HID, 1], F32) for l in range(3)]
    mw1_d = din("mlp_w1", [HID, 2 * HID], BF16)
    mw2_d = din("mlp_w2", [2 * HID, HID], BF16)
    mw3_d = din("mlp_w3", [HID, NCLS], BF16)
    mb1_d = din("mlp_b1", [2 * HID, 1], F32)
    mb2_d = din("mlp_b2", [HID, 1], F32)
    mb3_d = din("mlp_b3", [NCLS, 1], F32)
    out_d = nc.dram_tensor("out5", [NCLS, NPAD], F32,
                           kind="ExternalOutput").ap()
    dbg = bool(os.environ.get("KDBG"))
    if dbg:
        dbg_tab = nc.dram_tensor("dbg_tab", [NTOT, 64], F32,
                                 kind="ExternalOutput").ap()
        dbg_agg = nc.dram_tensor("dbg_agg", [NPAD, 64], F32,
                                 kind="ExternalOutput").ap()
        dbg_h = nc.dram_tensor("dbg_h", [128, NPAIR], F32,
                               kind="ExternalOutput").ap()

    rg = [list(range(NCORES))]

    with tile.TileContext(nc) as tc:
        with tc.tile_pool(name="sb", bufs=1) as sb, \
             tc.tile_pool(name="sb2", bufs=2) as sb2, \
             tc.tile_pool(name="hxp", bufs=2) as hxp, \
             tc.tile_pool(name="drp", bufs=1) as drp, \
             tc.tile_pool(name="mp", bufs=2) as mp, \
             tc.tile_pool(name="idxp", bufs=2) as idxp, \
             tc.tile_pool(name="psP", bufs=2, space="PSUM") as psP, \
             tc.tile_pool(name="psM", bufs=1, space="PSUM") as psM, \
             tc.tile_pool(name="dram", bufs=1, space="DRAM") as dram, \
             tc.tile_pool(name="dram2", bufs=2, space="DRAM") as dram2:

            # ---- persistent loads ----
            ew_f = sb.tile([128, NCH], F32, tag="ewf")
            EWCH = 1024
            for e0 in range(0, NCH, EWCH):
                e1 = min(e0 + EWCH, NCH)
                ewt = sb2.tile([128, EWCH], BF16, tag="ewtmp")
                nc.sync.dma_start(out=ewt[:, 0:e1 - e0], in_=ew_d[:, e0:e1])
                nc.vector.tensor_copy(out=ew_f[:, e0:e1],
                                      in_=ewt[:, 0:e1 - e0])
            cw_t = []
            for l in range(3):
                if l == 0:
                    t = sb.tile([IN_C, HID], BF16, tag=f"cw{l}")
                    nc.sync.dma_start(out=t[:], in_=cw_d[l][:])
                else:
                    # duplicated across parity halves so lhsT/rhs base
                    # partitions match for the odd-node projection
                    t = sb.tile([128, HID], BF16, tag=f"cw{l}")
                    nc.sync.dma_start(out=t[0:HID, :], in_=cw_d[l][:])
                    nc.sync.dma_start(out=t[HID:128, :], in_=cw_d[l][:])
                cw_t.append(t)
            cb_t, bng_t, bnb_t, cb2_t = [], [], [], []
            for l in range(3):
                tb = sb.tile([HID, 1], F32, tag=f"cb{l}")
                nc.sync.dma_start(out=tb[:], in_=cb_d[l][:])
                cb_t.append(tb)
                tb2 = sb.tile([128, 1], F32, tag=f"cbd{l}")
                nc.sync.dma_start(out=tb2[0:HID, :], in_=cb_d[l][:])
                nc.sync.dma_start(out=tb2[HID:128, :], in_=cb_d[l][:])
                cb2_t.append(tb2)
                tg = sb.tile([HID, 1], F32, tag=f"bng{l}")
                nc.sync.dma_start(out=tg[:], in_=bng_d[l][:])
                bng_t.append(tg)
                te = sb.tile([HID, 1], F32, tag=f"bnb{l}")
                nc.sync.dma_start(out=te[:], in_=bnb_d[l][:])
                bnb_t.append(te)
            mw1_t = sb.tile([128, 2 * HID], BF16, tag="mw1")
            nc.sync.dma_start(out=mw1_t[0:HID, :], in_=mw1_d[:])
            nc.sync.dma_start(out=mw1_t[HID:128, :], in_=mw1_d[:])
            mw2_t = sb.tile([2 * HID, HID], BF16, tag="mw2")
            nc.sync.dma_start(out=mw2_t[:], in_=mw2_d[:])
            mw3_t = sb.tile([HID, NCLS], BF16, tag="mw3")
            nc.sync.dma_start(out=mw3_t[:], in_=mw3_d[:])
            mb1_t = sb.tile([2 * HID, 1], F32, tag="mb1")
            nc.sync.dma_start(out=mb1_t[:], in_=mb1_d[:])
            mb2_t = sb.tile([HID, 1], F32, tag="mb2")
            nc.sync.dma_start(out=mb2_t[:], in_=mb2_d[:])
            mb3_t = sb.tile([NCLS, 1], F32, tag="mb3")
            nc.sync.dma_start(out=mb3_t[:], in_=mb3_d[:])
            zt = sb.tile([128, 1664], F32, tag="zero")
            nc.vector.memset(zt[:], 0.0)

            x_t = sb.tile([IN_C, NPAD], BF16, tag="hx0")
            nc.sync.dma_start(out=x_t[:], in_=x_t_d[:])

            # replicate wrapped indices [16, X] -> [128, X] in DRAM
            gidx_rep = dram.tile([128, NSLOT // 16], I16, tag="gidxrep")
            sidx_rep = dram.tile([128, NSLOT // 16], I16, tag="sidxrep")
            if LVL < 3:
                for r in range(8):
                    nc.sync.dma_start(
                        out=gidx_rep[16 * r:16 * (r + 1), :], in_=gidx_d[:])
                    nc.sync.dma_start(
                        out=sidx_rep[16 * r:16 * (r + 1), :], in_=sidx_d[:])

            h_cur = x_t              # layer0: plain feature-major
            for l in range(3):
                cdim = IN_C if l == 0 else HID
                # ---- projection -> compact node-major f32 table ----
                tab_in = dram2.tile([NPAD, 64], F32, tag="tabin")
                PB = 8
                if l == 0:
                    srcs = [(h_cur, 0, NTILES, tab_in[:])]
                else:
                    evens = tab_in[:].rearrange("(q i) f -> q i f", i=2)
                    srcs = [(h_cur, 0, NTILES // 2, evens[:, 0, :]),
                            (h_cur, HID, NTILES // 2, evens[:, 1, :])]
                for hsrc, prow, ntile, tview in srcs:
                    for g in range(0, ntile, PB):
                        gn = min(PB, ntile - g)
                        pp = psP.tile([128, PB * HID], F32, tag="proj",
                                      space="PSUM")
                        for j in range(gn):
                            kk = g + j
                            nc.tensor.matmul(
                                out=pp[:, j * HID:(j + 1) * HID],
                                lhsT=hsrc[prow:prow + cdim,
                                          kk * 128:(kk + 1) * 128],
                                rhs=cw_t[l][prow:prow + cdim, :],
                                start=True, stop=True)
                        stg = sb2.tile([128, PB, HID], F32, tag="stage")
                        nc.any.tensor_copy(
                            out=stg[:, 0:gn, :],
                            in_=pp[:, 0:gn * HID].rearrange(
                                "p (j f) -> p j f", j=gn))
                        tv = tview.rearrange("(g p) f -> g p f", p=128)
                        nc.sync.dma_start(
                            out=tv[g:g + gn].rearrange("g p f -> p g f"),
                            in_=stg[:, 0:gn, :])

                # ---- AllGather the projected table ----
                tab_full = dram2.tile([NTOT, 64], F32, tag="tabfull")
                if LVL < 4:
                    nc.gpsimd.collective_compute(
                        "AllGather", ALU.bypass, replica_groups=rg,
                        ins=[tab_in[:]], outs=[tab_full[:]])

                # ---- zero aggregate, gather + scatter-add ----
                agg = dram2.tile([NPAD + 128, 64], F32, tag="agg")
                av = agg[:].rearrange("(k p) f -> p k f", p=128)
                ZC = 1664 // 64
                for z0 in range(0, (NPAD + 128) // 128, ZC):
                    z1 = min(z0 + ZC, (NPAD + 128) // 128)
                    nc.sync.dma_start(
                        out=av[:, z0:z1, :],
                        in_=zt[:, 0:(z1 - z0) * 64].rearrange(
                            "p (k f) -> p k f", f=64))
                for wi in range(NWIN):
                    for (soff, nsl, runs) in segments[wi]:
                        segc = nsl // 128
                        if LVL < 3:
                            gi_t = idxp.tile([128, SEGC * 8], I16, tag="gi")
                            nc.sync.dma_start(
                                out=gi_t[:, 0:nsl // 16],
                                in_=gidx_rep[:, soff // 16:(soff + nsl) // 16])
                            si_t = idxp.tile([128, SEGC * 8], I16, tag="si")
                            nc.sync.dma_start(
                                out=si_t[:, 0:nsl // 16],
                                in_=sidx_rep[:, soff // 16:(soff + nsl) // 16])
                            m_t = mp.tile([128, SEGC, 64], F32, tag="mt")
                            nc.gpsimd.dma_gather(
                                out_ap=m_t[:, 0:segc, :],
                                in_ap=tab_full[wi * WINR:(wi + 1) * WINR, :],
                                idxs_ap=gi_t[:, 0:nsl // 16],
                                num_idxs=nsl, num_idxs_reg=nsl,
                                elem_size=64, single_packet=False)
                        if LVL < 2:
                            a0, a1 = bass.broadcast_tensor_aps(
                                m_t[:, 0:segc, :],
                                ew_f[:, soff // 128:soff // 128 + segc]
                                .rearrange("p (k a) -> p k a", a=1))
                            nc.vector.tensor_tensor(
                                out=m_t[:, 0:segc, :], in0=a0, in1=a1,
                                op=ALU.mult)
                        if LVL < 1:
                            for k in runs:
                                ro, rl = int(run_off[wi][k]), int(RL[wi][k])
                                for p0 in range(0, rl, SCATMAX):
                                    pn = min(SCATMAX, rl - p0)
                                    lo = ro - soff + p0
                                    nc.gpsimd.dma_scatter_add(
                                        out_ap=agg[:],
                                        in_ap=m_t[:, lo // 128:
                                                  (lo + pn) // 128, :],
                                        idxs_ap=si_t[:, lo // 16:
                                                     (lo + pn) // 16],
                                        num_idxs=pn, num_idxs_reg=pn,
                                        elem_size=64, single_packet=False)

                # ---- drain: f32 node-major -> bf16 pair-rows -> xbar T ----
                if dbg and l == 0:
                    nc.sync.dma_start(out=dbg_tab[:], in_=tab_full[:])
                    nc.sync.dma_start(out=dbg_agg[:], in_=agg[0:NPAD, :])
                af = drp.tile([128, NTILES, 64], F32, tag="af")
                nc.sync.dma_start(out=af[:], in_=av[:, 0:NTILES, :])
                abf = drp.tile([128, NTILES, 64], BF16, tag="abf")
                nc.vector.tensor_copy(out=abf[:], in_=af[:])
                tmp = dram2.tile([NPAIR, 128], BF16, tag="ptmp")
                nc.sync.dma_start(
                    out=tmp[:].rearrange("(k q) (i f) -> (q i) k f",
                                         q=64, i=2),
                    in_=abf[:])
                h_new = hxp.tile([128, NPAIR], BF16, tag="hx")
                nc.sync.dma_start_transpose(out=h_new[:], in_=tmp[:])
                # bias + relu (bias duplicated across parity halves)
                nc.vector.tensor_scalar(
                    out=h_new[:], in0=h_new[:], scalar1=cb2_t[l][:],
                    scalar2=0.0, op0=ALU.add, op1=ALU.max)

                # ---- BN stats (global, pad-row corrected) ----
                scr = drp.tile([128, NPAIR], BF16, tag="scr")
                st2 = sb.tile([128, 2], F32, tag="st2")
                nc.scalar.activation(out=scr[:], in_=h_new[:],
                                     func=ACTF.Copy, accum_out=st2[:, 0:1])
                nc.scalar.activation(out=scr[:], in_=h_new[:],
                                     func=ACTF.Square, accum_out=st2[:, 1:2])
                fold = sb.tile([HID, 2], F32, tag="fold")
                nc.sync.dma_start(out=fold[:], in_=st2[HID:128, :])
                pb = sb.tile([HID, 1], F32, tag="pb")
                nc.vector.tensor_scalar(out=pb[:], in0=cb_t[l][:],
                                        scalar1=0.0, scalar2=None,
                                        op0=ALU.max)
                pb2 = sb.tile([HID, 1], F32, tag="pb2")
                nc.vector.tensor_tensor(out=pb2[:], in0=pb[:], in1=pb[:],
                                        op=ALU.mult)
                sc = sb.tile([HID, 2], F32, tag="statsc")
                nc.vector.tensor_scalar(out=sc[:, 0:1], in0=pb[:],
                                        scalar1=-NPADDING, scalar2=None,
                                        op0=ALU.mult)
                nc.vector.tensor_scalar(out=sc[:, 1:2], in0=pb2[:],
                                        scalar1=-NPADDING, scalar2=None,
                                        op0=ALU.mult)
                nc.vector.tensor_tensor(out=sc[:], in0=sc[:],
                                        in1=st2[0:HID, :], op=ALU.add)
                nc.vector.tensor_tensor(out=sc[:], in0=sc[:],
                                        in1=fold[:], op=ALU.add)
                stats_in = dram.tile([HID, 2], F32, tag=f"stin{l}")
                stats_out = dram.tile([HID, 2], F32, tag=f"stout{l}")
                st_sb = sb.tile([HID, 2], F32, tag="stsb")
                if LVL < 4:
                    nc.sync.dma_start(out=stats_in[:], in_=sc[:])
                    nc.gpsimd.collective_compute(
                        "AllReduce", ALU.add, replica_groups=rg,
                        ins=[stats_in[:]], outs=[stats_out[:]])
                    nc.sync.dma_start(out=st_sb[:], in_=stats_out[:])
                else:
                    nc.vector.tensor_copy(out=st_sb[:], in_=sc[:])
                mt = sb.tile([HID, 1], F32, tag="mt1")
                nc.vector.tensor_scalar(out=mt[:], in0=st_sb[:, 0:1],
                                        scalar1=INVN, scalar2=None,
                                        op0=ALU.mult)
                vt = sb.tile([HID, 1], F32, tag="vt")
                nc.vector.tensor_scalar(out=vt[:], in0=st_sb[:, 1:2],
                                        scalar1=INVN, scalar2=None,
                                        op0=ALU.mult)
                msq = sb.tile([HID, 1], F32, tag="msq")
                nc.vector.tensor_tensor(out=msq[:], in0=mt[:], in1=mt[:],
                                        op=ALU.mult)
                nc.vector.tensor_tensor(out=vt[:], in0=vt[:], in1=msq[:],
                                        op=ALU.subtract)
                nc.vector.tensor_scalar(out=vt[:], in0=vt[:], scalar1=EPS,
                                        scalar2=None, op0=ALU.add)
                sqv = sb.tile([HID, 1], F32, tag="sqv")
                nc.scalar.activation(out=sqv[:], in_=vt[:], func=ACTF.Sqrt)
                rstd = sb.tile([HID, 1], F32, tag="rstd")
                nc.vector.reciprocal(out=rstd[:], in_=sqv[:])
                s_t = sb.tile([128, 1], F32, tag="sT")
                nc.vector.tensor_tensor(out=s_t[0:HID, :], in0=bng_t[l][:],
                                        in1=rstd[:], op=ALU.mult)
                t_t = sb.tile([128, 1], F32, tag="tT")
                nc.vector.tensor_tensor(out=t_t[0:HID, :], in0=mt[:],
                                        in1=s_t[0:HID, :], op=ALU.mult)
                nc.vector.tensor_tensor(out=t_t[0:HID, :], in0=bnb_t[l][:],
                                        in1=t_t[0:HID, :], op=ALU.subtract)
                nc.sync.dma_start(out=s_t[HID:128, :], in_=s_t[0:HID, :])
                nc.sync.dma_start(out=t_t[HID:128, :], in_=t_t[0:HID, :])
                nc.vector.tensor_scalar(out=h_new[:], in0=h_new[:],
                                        scalar1=s_t[:], scalar2=t_t[:],
                                        op0=ALU.mult, op1=ALU.add)
                if dbg and l == 0:
                    dh1 = drp.tile([128, NPAIR], F32, tag="dh1")
                    nc.vector.tensor_copy(out=dh1[:], in_=h_new[:])
                    nc.sync.dma_start(out=dbg_h[:], in_=dh1[:])
                h_cur = h_new

            # ---- MLP head (parity-interleaved feature-major) ----
            MB = 512
            ov = out_d[:].rearrange("c (q i) -> c q i", i=2)
            for par in range(2):
                hsec = h_cur[par * HID:(par + 1) * HID, :]
                for s0 in range(0, NPAIR, MB):
                    s1 = min(s0 + MB, NPAIR)
                    n = s1 - s0
                    p1 = psM.tile([2 * HID, MB], F32, tag="mp1", space="PSUM")
                    nc.tensor.matmul(out=p1[:, 0:n],
                                     lhsT=mw1_t[par * HID:(par + 1) * HID, :],
                                     rhs=hsec[:, s0:s1],
                                     start=True, stop=True)
                    a1 = sb2.tile([2 * HID, MB], BF16, tag="a1")
                    nc.scalar.activation(out=a1[:, 0:n], in_=p1[:, 0:n],
                                         func=ACTF.Gelu, bias=mb1_t[:])
                    p2 = psM.tile([HID, MB], F32, tag="mp2", space="PSUM")
                    nc.tensor.matmul(out=p2[:, 0:n], lhsT=mw2_t[:],
                                     rhs=a1[:, 0:n], start=True, stop=True)
                    a2 = sb2.tile([HID, MB], BF16, tag="a2")
                    nc.scalar.activation(out=a2[:, 0:n], in_=p2[:, 0:n],
                                         func=ACTF.Gelu, bias=mb2_t[:])
                    p3 = psM.tile([NCLS, MB], F32, tag="mp3", space="PSUM")
                    nc.tensor.matmul(out=p3[:, 0:n], lhsT=mw3_t[:],
                                     rhs=a2[:, 0:n], start=True, stop=True)
                    ob = sb2.tile([NCLS, MB], F32, tag="ob")
                    nc.vector.tensor_scalar(out=ob[:, 0:n], in0=p3[:, 0:n],
                                            scalar1=mb3_t[:], scalar2=None,
                                            op0=ALU.add)
                    nc.sync.dma_start(out=ov[:, s0:s1, par],
                                      in_=ob[:, 0:n])
    nc.compile()
    return nc


def kernel(x, edge_index, edge_attr,
           conv_w0, conv_b0, conv_w1, conv_b1, conv_w2, conv_b2,
           bn_g0, bn_be0, bn_g1, bn_be1, bn_g2, bn_be2,
           mlp_w1, mlp_b1, mlp_w2, mlp_b2, mlp_w3, mlp_b3):
    x = np.asarray(x)
    N, in_c = x.shape
    hid = np.asarray(conv_w0).shape[1]
    ncls = np.asarray(mlp_w3).shape[1]
    cfg = _cfg(N, in_c, hid, ncls)

    edge_index = np.asarray(edge_index)
    plan = _plan(edge_index, cfg)
    per_core = _preprocess(x, edge_index, edge_attr, cfg, plan)

    bf = ml_dtypes.bfloat16
    common = dict(
        conv_w0=np.asarray(conv_w0).astype(bf),
        conv_w1=np.asarray(conv_w1).astype(bf),
        conv_w2=np.asarray(conv_w2).astype(bf),
        conv_b0=np.asarray(conv_b0, dtype=np.float32).reshape(-1, 1),
        conv_b1=np.asarray(conv_b1, dtype=np.float32).reshape(-1, 1),
        conv_b2=np.asarray(conv_b2, dtype=np.float32).reshape(-1, 1),
        bn_g0=np.asarray(bn_g0, dtype=np.float32).reshape(-1, 1),
        bn_g1=np.asarray(bn_g1, dtype=np.float32).reshape(-1, 1),
        bn_g2=np.asarray(bn_g2, dtype=np.float32).reshape(-1, 1),
        bn_be0=np.asarray(bn_be0, dtype=np.float32).reshape(-1, 1),
        bn_be1=np.asarray(bn_be1, dtype=np.float32).reshape(-1, 1),
        bn_be2=np.asarray(bn_be2, dtype=np.float32).reshape(-1, 1),
        mlp_w1=np.asarray(mlp_w1).astype(bf),
        mlp_w2=np.asarray(mlp_w2).astype(bf),
        mlp_w3=np.asarray(mlp_w3).astype(bf),
        mlp_b1=np.asarray(mlp_b1, dtype=np.float32).reshape(-1, 1),
        mlp_b2=np.asarray(mlp_b2, dtype=np.float32).reshape(-1, 1),
        mlp_b3=np.asarray(mlp_b3, dtype=np.float32).reshape(-1, 1),
    )
    in_maps = []
    for c in range(NCORES):
        m = dict(common)
        m["x_t"] = per_core[c]["x_t"]
        m["g_idx"] = per_core[c]["g_idx"]
        m["s_idx"] = per_core[c]["s_idx"]
        m["ew_s"] = per_core[c]["ew_s"]
        in_maps.append(m)

    nc = _build(cfg, plan)
    nc.m = get_hw_module(nc.m)
    res = bass_utils.run_bass_kernel_spmd(
        nc, in_maps, core_ids=list(range(NCORES)))

    kernel._last_res = res
    kernel._last_nc = nc
    kernel._last_in_maps = in_maps
    out = np.empty((N, cfg["NCLS"]), dtype=np.float32)
    SHARD = cfg["SHARD"]
    for c in range(NCORES):
        out[c * SHARD:(c + 1) * SHARD] = res.results[c]["out5"][:, :SHARD].T
    return out


# revision 19
# speedup vs baseline: 9.9690x; 1.0592x over previous
"""Trainium2 Bass kernel for MineralDepositGCN (3x GCNConv+BN + MLP head).

Strategy (8 NeuronCores, SPMD single program), per the sharding hint:
  - Nodes sharded by range: core c owns nodes [c*12500, (c+1)*12500),
    padded to NPAD=12800. Edges owned by their DST core, so aggregation is
    device-local; the halo exchange is an AllGather of projected features.
  - Per layer: project own shard h@W into a compact node-major f32 table
    [NPAD, 64] (256B rows), AllGather -> [8*NPAD, 64] in DRAM.
  - Messages fetched with dma_gather (256B rows, all-useful f32). int16
    gather indices reach 32767 rows only, so edges are bucketed by
    src-core-pair "window" (4 windows x 25600 rows), gathered from a
    sliced table view.
  - Scatter-add via dma_scatter_add (SDMA CCE f32 accumulate) into a
    DRAM aggregate [NPAD, 64]. CCE loses concurrent duplicate updates
    within one call, so edges are split into conflict-free runs: run
    (w, k) holds each dst's k-th edge from window w; runs execute as
    separate (serialized) scatter calls. Gathers batch several runs per
    call; edge-weight scaling is one in-place broadcast multiply per
    segment. This keeps the whole edge stage at ~260 instructions/layer
    (the backend executes ~12k instructions/s, so instruction count
    dominates the runtime).
  - Aggregate drain: f32 node-major -> bf16 [6400, 128] (two nodes per
    row) -> hardware xbar dma transpose -> parity-interleaved
    feature-major h [128=(feat,parity), 6400]. Projection/BN/MLP all
    operate on this layout at unchanged instruction counts.
  - BN stats via 2 accum passes + parity fold + tiny AllReduce with
    closed-form pad-row correction.
"""
import os
import numpy as np
import ml_dtypes

from concourse import bass, bacc, tile, mybir
from concourse import bass_utils
from concourse.bass_interp import get_hw_module

BF16 = mybir.dt.bfloat16
F32 = mybir.dt.float32
I16 = mybir.dt.int16
ALU = mybir.AluOpType
ACTF = mybir.ActivationFunctionType

NCORES = 8
EPS = 1e-5
SEGMAX = 13056          # max gather-segment slots (128-mult)
SCATMAX = 8064          # max slots per dma_scatter_add call (2 desc/idx, <16384)


def _cfg(n_nodes, in_c, hid, ncls):
    shard = n_nodes // NCORES
    npad = ((shard + 511) // 512) * 512
    return dict(
        N=n_nodes, IN_C=in_c, HID=hid, NCLS=ncls,
        SHARD=shard, NPAD=npad,
        NTILES=npad // 128,
        NTOT=npad * NCORES,
        WINR=2 * npad,
        NWIN=NCORES // 2,
    )


def _plan(edge_index, cfg):
    """Conflict-free run structure shared by all cores.

    Edges keyed by (window w, rank k) where k = occurrence index of the
    edge's dst within window w on its owner core. Run (w, k) has
    RL[w][k] = 128*ceil(max_core count/128) slots. Runs are packed in
    (w, k) order into gather segments of <= SEGMAX slots.
    """
    SHARD, NWIN, NPAD = cfg["SHARD"], cfg["NWIN"], cfg["NPAD"]
    src = edge_index[0].astype(np.int64)
    dst = edge_index[1].astype(np.int64)
    d_owner = np.minimum(dst // SHARD, NCORES - 1)
    d_local = dst - d_owner * SHARD
    s_owner = np.minimum(src // SHARD, NCORES - 1)
    w = s_owner // 2

    # rank of each edge within its (owner, w, dst) group
    key = (d_owner * NWIN + w) * SHARD + d_local
    order = np.argsort(key, kind="stable")
    sk = key[order]
    new = np.ones(len(sk), dtype=bool)
    new[1:] = sk[1:] != sk[:-1]
    idxs = np.arange(len(sk))
    starts = idxs[new]
    grp_start = starts[np.cumsum(new) - 1]
    rank_sorted = idxs - grp_start
    rank = np.empty(len(sk), dtype=np.int64)
    rank[order] = rank_sorted

    KMAX = int(rank.max()) + 1
    cnt = np.zeros((NCORES, NWIN, KMAX), dtype=np.int64)
    np.add.at(cnt.reshape(-1), (d_owner * NWIN + w) * KMAX + rank, 1)
    cmax = cnt.max(axis=0)                       # [NWIN, KMAX]
    RL = ((cmax + 127) // 128) * 128
    run_off = np.zeros((NWIN, KMAX), dtype=np.int64)
    segments = []                                # per window
    off = 0
    for wi in range(NWIN):
        segs = []
        cur_off, cur_n, cur_runs = off, 0, []
        for k in range(KMAX):
            rl = int(RL[wi, k])
            if rl == 0:
                continue
            if cur_n + rl > SEGMAX and cur_n > 0:
                segs.append((cur_off, cur_n, cur_runs))
                cur_off, cur_n, cur_runs = off, 0, []
            run_off[wi, k] = off
            cur_runs.append(k)
            cur_n += rl
            off += rl
        if cur_n > 0:
            segs.append((cur_off, cur_n, cur_runs))
        segments.append(segs)
    NSLOT = int(off)
    return dict(rank=rank, w=w, d_owner=d_owner, d_local=d_local,
                RL=RL, run_off=run_off, segments=segments,
                NSLOT=NSLOT, KMAX=KMAX)


def _preprocess(x, edge_index, edge_attr, cfg, plan):
    SHARD, NPAD, NWIN = cfg["SHARD"], cfg["NPAD"], cfg["NWIN"]
    WINR = cfg["WINR"]
    NSLOT = plan["NSLOT"]
    src = edge_index[0].astype(np.int64)
    ew = np.asarray(edge_attr, dtype=np.float32)
    s_owner = np.minimum(src // SHARD, NCORES - 1)
    s_local = src - s_owner * SHARD
    gidx_all = (s_owner * NPAD + s_local) - plan["w"] * WINR
    JUNK = NPAD                                  # scatter junk row

    bf = ml_dtypes.bfloat16
    per_core = []
    for c in range(NCORES):
        m = plan["d_owner"] == c
        wi = plan["w"][m]
        k = plan["rank"][m]
        gi = gidx_all[m]
        dl = plan["d_local"][m]
        we = ew[m]
        # position within run: order by (w, k, dst)
        order = np.lexsort((dl, k, wi))
        wi, k, gi, dl, we = (wi[order], k[order], gi[order], dl[order],
                             we[order])
        runkey = wi * plan["KMAX"] + k
        new = np.ones(len(runkey), dtype=bool)
        new[1:] = runkey[1:] != runkey[:-1]
        idxs = np.arange(len(runkey))
        starts = idxs[new]
        pos = idxs - starts[np.cumsum(new) - 1]
        slot = plan["run_off"][wi, k] + pos

        gidx16 = np.zeros(NSLOT, dtype=np.int16)
        sidx16 = np.full(NSLOT, JUNK, dtype=np.int16)
        ewb = np.zeros(NSLOT, dtype=np.float32)
        gidx16[slot] = gi.astype(np.int16)
        sidx16[slot] = dl.astype(np.int16)
        ewb[slot] = we

        per_core.append(dict(
            g_idx=gidx16.reshape(NSLOT // 16, 16).T.copy(),
            s_idx=sidx16.reshape(NSLOT // 16, 16).T.copy(),
            ew_s=ewb.reshape(NSLOT // 128, 128).T.astype(bf),
        ))

    for c in range(NCORES):
        xs = np.zeros((cfg["IN_C"], NPAD), dtype=np.float32)
        xs[:, :SHARD] = np.asarray(x[c * SHARD:(c + 1) * SHARD]).T
        per_core[c]["x_t"] = xs.astype(bf)
    return per_core


def _build(cfg, plan):
    IN_C, HID, NCLS = cfg["IN_C"], cfg["HID"], cfg["NCLS"]
    NPAD, NTILES = cfg["NPAD"], cfg["NTILES"]
    NTOT, WINR, NWIN = cfg["NTOT"], cfg["WINR"], cfg["NWIN"]
    NSLOT = plan["NSLOT"]
    RL, run_off, segments = plan["RL"], plan["run_off"], plan["segments"]
    NPAIR = NPAD // 2
    NPADDING = float(NCORES * NPAD - cfg["N"])
    INVN = 1.0 / cfg["N"]
    SEGC = SEGMAX // 128
    LVL = int(os.environ.get("KLEVEL", "0"))
    NCH = NSLOT // 128

    nc = bacc.Bacc("TRN2", target_bir_lowering=False, debug=False,
                   num_devices=NCORES)

    def din(name, shape, dt):
        return nc.dram_tensor(name, shape, dt, kind="ExternalInput").ap()

    x_t_d = din("x_t", [IN_C, NPAD], BF16)
    gidx_d = din("g_idx", [16, NSLOT // 16], I16)
    sidx_d = din("s_idx", [16, NSLOT // 16], I16)
    ew_d = din("ew_s", [128, NCH], BF16)
    cw_d = [din(f"conv_w{l}", [IN_C if l == 0 else HID, HID], BF16)
            for l in range(3)]
    cb_d = [din(f"conv_b{l}", [HID, 1], F32) for l in range(3)]
    bng_d = [din(f"bn_g{l}", [HID, 1], F32) for l in range(3)]
    bnb_d = [din(f"bn_be{l}", [HID, 1], F32) for l in range(3)]
    mw1_d = din("mlp_w1", [HID, 2 * HID], BF16)
    mw2_d = din("mlp_w2", [2 * HID, HID], BF16)
    mw3_d = din("mlp_w3", [HID, NCLS], BF16)
    mb1_d = din("mlp_b1", [2 * HID, 1], F32)
    mb2_d = din("mlp_b2", [HID, 1], F32)
    mb3_d = din("mlp_b3", [NCLS, 1], F32)
    out_d = nc.dram_tensor("out5", [NCLS, NPAD], F32,
                           kind="ExternalOutput").ap()
    dbg = bool(os.environ.get("KDBG"))
    if dbg:
        dbg_tab = nc.dram_tensor("dbg_tab", [NTOT, 64], F32,
                                 kind="ExternalOutput").ap()
        dbg_agg = nc.dram_tensor("dbg_agg", [NPAD, 64], F32,
                                 kind="ExternalOutput").ap()
        dbg_h = nc.dram_tensor("dbg_h", [128, NPAIR], F32,
                               kind="ExternalOutput").ap()

    rg = [list(range(NCORES))]

    with tile.TileContext(nc) as tc:
        with tc.tile_pool(name="sb", bufs=1) as sb, \
             tc.tile_pool(name="sb2", bufs=2) as sb2, \
             tc.tile_pool(name="hxp", bufs=2) as hxp, \
             tc.tile_pool(name="drp", bufs=1) as drp, \
             tc.tile_pool(name="mp", bufs=2) as mp, \
             tc.tile_pool(name="idxp", bufs=2) as idxp, \
             tc.tile_pool(name="psP", bufs=2, space="PSUM") as psP, \
             tc.tile_pool(name="psM", bufs=1, space="PSUM") as psM, \
             tc.tile_pool(name="dram", bufs=1, space="DRAM") as dram, \
             tc.tile_pool(name="dram2", bufs=2, space="DRAM") as dram2:

            # ---- persistent loads ----
            ew_f = sb.tile([128, NCH], F32, tag="ewf")
            EWCH = 1024
            for e0 in range(0, NCH, EWCH):
                e1 = min(e0 + EWCH, NCH)
                ewt = sb2.tile([128, EWCH], BF16, tag="ewtmp")
                nc.sync.dma_start(out=ewt[:, 0:e1 - e0], in_=ew_d[:, e0:e1])
                nc.vector.tensor_copy(out=ew_f[:, e0:e1],
                                      in_=ewt[:, 0:e1 - e0])
            cw_t = []
            for l in range(3):
                if l == 0:
                    t = sb.tile([IN_C, HID], BF16, tag=f"cw{l}")
                    nc.sync.dma_start(out=t[:], in_=cw_d[l][:])
                else:
                    # duplicated across parity halves so lhsT/rhs base
                    # partitions match for the odd-node projection
                    t = sb.tile([128, HID], BF16, tag=f"cw{l}")
                    nc.sync.dma_start(out=t[0:HID, :], in_=cw_d[l][:])
                    nc.sync.dma_start(out=t[HID:128, :], in_=cw_d[l][:])
                cw_t.append(t)
            cb_t, bng_t, bnb_t, cb2_t = [], [], [], []
            for l in range(3):
                tb = sb.tile([HID, 1], F32, tag=f"cb{l}")
                nc.sync.dma_start(out=tb[:], in_=cb_d[l][:])
                cb_t.append(tb)
                tb2 = sb.tile([128, 1], F32, tag=f"cbd{l}")
                nc.sync.dma_start(out=tb2[0:HID, :], in_=cb_d[l][:])
                nc.sync.dma_start(out=tb2[HID:128, :], in_=cb_d[l][:])
                cb2_t.append(tb2)
                tg = sb.tile([HID, 1], F32, tag=f"bng{l}")
                nc.sync.dma_start(out=tg[:], in_=bng_d[l][:])
                bng_t.append(tg)
                te = sb.tile([HID, 1], F32, tag=f"bnb{l}")
                nc.sync.dma_start(out=te[:], in_=bnb_d[l][:])
                bnb_t.append(te)
            mw1_t = sb.tile([128, 2 * HID], BF16, tag="mw1")
            nc.sync.dma_start(out=mw1_t[0:HID, :], in_=mw1_d[:])
            nc.sync.dma_start(out=mw1_t[HID:128, :], in_=mw1_d[:])
            mw2_t = sb.tile([2 * HID, HID], BF16, tag="mw2")
            nc.sync.dma_start(out=mw2_t[:], in_=mw2_d[:])
            mw3_t = sb.tile([HID, NCLS], BF16, tag="mw3")
            nc.sync.dma_start(out=mw3_t[:], in_=mw3_d[:])
            mb1_t = sb.tile([2 * HID, 1], F32, tag="mb1")
            nc.sync.dma_start(out=mb1_t[:], in_=mb1_d[:])
            mb2_t = sb.tile([HID, 1], F32, tag="mb2")
            nc.sync.dma_start(out=mb2_t[:], in_=mb2_d[:])
            mb3_t = sb.tile([NCLS, 1], F32, tag="mb3")
            nc.sync.dma_start(out=mb3_t[:], in_=mb3_d[:])
            zt = sb.tile([128, 1664], F32, tag="zero")
            nc.vector.memset(zt[:], 0.0)

            x_t = sb.tile([IN_C, NPAD], BF16, tag="hx0")
            nc.sync.dma_start(out=x_t[:], in_=x_t_d[:])

            # replicate wrapped indices [16, X] -> [128, X] in DRAM
            gidx_rep = dram.tile([128, NSLOT // 16], I16, tag="gidxrep")
            sidx_rep = dram.tile([128, NSLOT // 16], I16, tag="sidxrep")
            if LVL < 3:
                for r in range(8):
                    nc.sync.dma_start(
                        out=gidx_rep[16 * r:16 * (r + 1), :], in_=gidx_d[:])
                    nc.sync.dma_start(
                        out=sidx_rep[16 * r:16 * (r + 1), :], in_=sidx_d[:])

            h_cur = x_t              # layer0: plain feature-major
            for l in range(3):
                cdim = IN_C if l == 0 else HID
                # ---- projection -> compact node-major f32 table ----
                tab_in = dram2.tile([NPAD, 64], F32, tag="tabin")
                PB = 8
                if l == 0:
                    srcs = [(h_cur, 0, NTILES, tab_in[:])]
                else:
                    evens = tab_in[:].rearrange("(q i) f -> q i f", i=2)
                    srcs = [(h_cur, 0, NTILES // 2, evens[:, 0, :]),
                            (h_cur, HID, NTILES // 2, evens[:, 1, :])]
                for hsrc, prow, ntile, tview in srcs:
                    for g in range(0, ntile, PB):
                        gn = min(PB, ntile - g)
                        pp = psP.tile([128, PB * HID], F32, tag="proj",
                                      space="PSUM")
                        for j in range(gn):
                            kk = g + j
                            nc.tensor.matmul(
                                out=pp[:, j * HID:(j + 1) * HID],
                                lhsT=hsrc[prow:prow + cdim,
                                          kk * 128:(kk + 1) * 128],
                                rhs=cw_t[l][prow:prow + cdim, :],
                                start=True, stop=True)
                        stg = sb2.tile([128, PB, HID], F32, tag="stage")
                        nc.any.tensor_copy(
                            out=stg[:, 0:gn, :],
                            in_=pp[:, 0:gn * HID].rearrange(
                                "p (j f) -> p j f", j=gn))
                        tv = tview.rearrange("(g p) f -> g p f", p=128)
                        nc.sync.dma_start(
                            out=tv[g:g + gn].rearrange("g p f -> p g f"),
                            in_=stg[:, 0:gn, :])

                # ---- AllGather the projected table ----
                tab_full = dram2.tile([NTOT, 64], F32, tag="tabfull")
                if LVL < 4:
                    nc.gpsimd.collective_compute(
                        "AllGather", ALU.bypass, replica_groups=rg,
                        ins=[tab_in[:]], outs=[tab_full[:]])

                # ---- zero aggregate, gather + scatter-add ----
                agg = dram2.tile([NPAD + 128, 64], F32, tag="agg")
                av = agg[:].rearrange("(k p) f -> p k f", p=128)
                ZC = 1664 // 64
                for z0 in range(0, (NPAD + 128) // 128, ZC):
                    z1 = min(z0 + ZC, (NPAD + 128) // 128)
                    nc.sync.dma_start(
                        out=av[:, z0:z1, :],
                        in_=zt[:, 0:(z1 - z0) * 64].rearrange(
                            "p (k f) -> p k f", f=64))
                for wi in range(NWIN):
                    for (soff, nsl, runs) in segments[wi]:
                        segc = nsl // 128
                        if LVL < 3:
                            gi_t = idxp.tile([128, SEGC * 8], I16, tag="gi")
                            nc.sync.dma_start(
                                out=gi_t[:, 0:nsl // 16],
                                in_=gidx_rep[:, soff // 16:(soff + nsl) // 16])
                            si_t = idxp.tile([128, SEGC * 8], I16, tag="si")
                            nc.sync.dma_start(
                                out=si_t[:, 0:nsl // 16],
                                in_=sidx_rep[:, soff // 16:(soff + nsl) // 16])
                            m_t = mp.tile([128, SEGC, 64], F32, tag="mt")
                            nc.gpsimd.dma_gather(
                                out_ap=m_t[:, 0:segc, :],
                                in_ap=tab_full[wi * WINR:(wi + 1) * WINR, :],
                                idxs_ap=gi_t[:, 0:nsl // 16],
                                num_idxs=nsl, num_idxs_reg=nsl,
                                elem_size=64, single_packet=False)
                        if LVL < 2:
                            a0, a1 = bass.broadcast_tensor_aps(
                                m_t[:, 0:segc, :],
                                ew_f[:, soff // 128:soff // 128 + segc]
                                .rearrange("p (k a) -> p k a", a=1))
                            nc.vector.tensor_tensor(
                                out=m_t[:, 0:segc, :], in0=a0, in1=a1,
                                op=ALU.mult)
                        if LVL < 1:
                            for k in runs:
                                ro, rl = int(run_off[wi][k]), int(RL[wi][k])
                                for p0 in range(0, rl, SCATMAX):
                                    pn = min(SCATMAX, rl - p0)
                                    lo = ro - soff + p0
                                    nc.gpsimd.dma_scatter_add(
                                        out_ap=agg[:],
                                        in_ap=m_t[:, lo // 128:
                                                  (lo + pn) // 128, :],
                                        idxs_ap=si_t[:, lo // 16:
                                                     (lo + pn) // 16],
                                        num_idxs=pn, num_idxs_reg=pn,
                                        elem_size=64, single_packet=False)

                # ---- drain: f32 node-major -> bf16 pair-rows -> xbar T ----
                if dbg and l == 0:
                    nc.sync.dma_start(out=dbg_tab[:], in_=tab_full[:])
                    nc.sync.dma_start(out=dbg_agg[:], in_=agg[0:NPAD, :])
                af = drp.tile([128, NTILES, 64], F32, tag="af")
                nc.sync.dma_start(out=af[:], in_=av[:, 0:NTILES, :])
                abf = drp.tile([128, NTILES, 64], BF16, tag="abf")
                nc.vector.tensor_copy(out=abf[:], in_=af[:])
                tmp = dram2.tile([NPAIR, 128], BF16, tag="ptmp")
                nc.sync.dma_start(
                    out=tmp[:].rearrange("(k q) (i f) -> (q i) k f",
                                         q=64, i=2),
                    in_=abf[:])
                h_new = hxp.tile([128, NPAIR], BF16, tag="hx")
                nc.sync.dma_start_transpose(out=h_new[:], in_=tmp[:])
                # bias + relu (bias duplicated across parity halves)
                nc.vector.tensor_scalar(
                    out=h_new[:], in0=h_new[:], scalar1=cb2_t[l][:],
                    scalar2=0.0, op0=ALU.add, op1=ALU.max)

                # ---- BN stats (global, pad-row corrected) ----
                scr = drp.tile([128, NPAIR], BF16, tag="scr")
                st2 = sb.tile([128, 2], F32, tag="st2")
                nc.scalar.activation(out=scr[:], in_=h_new[:],
                                     func=ACTF.Copy, accum_out=st2[:, 0:1])
                nc.scalar.activation(out=scr[:], in_=h_new[:],
                                     func=ACTF.Square, accum_out=st2[:, 1:2])
                fold = sb.tile([HID, 2], F32, tag="fold")
                nc.sync.dma_start(out=fold[:], in_=st2[HID:128, :])
                pb = sb.tile([HID, 1], F32, tag="pb")
                nc.vector.tensor_scalar(out=pb[:], in0=cb_t[l][:],
                                        scalar1=0.0, scalar2=None,
                                        op0=ALU.max)
                pb2 = sb.tile([HID, 1], F32, tag="pb2")
                nc.vector.tensor_tensor(out=pb2[:], in0=pb[:], in1=pb[:],
                                        op=ALU.mult)
                sc = sb.tile([HID, 2], F32, tag="statsc")
                nc.vector.tensor_scalar(out=sc[:, 0:1], in0=pb[:],
                                        scalar1=-NPADDING, scalar2=None,
                                        op0=ALU.mult)
                nc.vector.tensor_scalar(out=sc[:, 1:2], in0=pb2[:],
                                        scalar1=-NPADDING, scalar2=None,
                                        op0=ALU.mult)
                nc.vector.tensor_tensor(out=sc[:], in0=sc[:],
                                        in1=st2[0:HID, :], op=ALU.add)
                nc.vector.tensor_tensor(out=sc[:], in0=sc[:],
                                        in1=fold[:], op=ALU.add)
                stats_in = dram.tile([HID, 2], F32, tag=f"stin{l}")
                stats_out = dram.tile([HID, 2], F32, tag=f"stout{l}")
                st_sb = sb.tile([HID, 2], F32, tag="stsb")
                if LVL < 4:
                    nc.sync.dma_start(out=stats_in[:], in_=sc[:])
                    nc.gpsimd.collective_compute(
                        "AllReduce", ALU.add, replica_groups=rg,
                        ins=[stats_in[:]], outs=[stats_out[:]])
                    nc.sync.dma_start(out=st_sb[:], in_=stats_out[:])
                else:
                    nc.vector.tensor_copy(out=st_sb[:], in_=sc[:])
                mt = sb.tile([HID, 1], F32, tag="mt1")
                nc.vector.tensor_scalar(out=mt[:], in0=st_sb[:, 0:1],
                                        scalar1=INVN, scalar2=None,
                                        op0=ALU.mult)
                vt = sb.tile([HID, 1], F32, tag="vt")
                nc.vector.tensor_scalar(out=vt[:], in0=st_sb[:, 1:2],
                                        scalar1=INVN, scalar2=None,
                                        op0=ALU.mult)
                msq = sb.tile([HID, 1], F32, tag="msq")
                nc.vector.tensor_tensor(out=msq[:], in0=mt[:], in1=mt[:],
                                        op=ALU.mult)
                nc.vector.tensor_tensor(out=vt[:], in0=vt[:], in1=msq[:],
                                        op=ALU.subtract)
                nc.vector.tensor_scalar(out=vt[:], in0=vt[:], scalar1=EPS,
                                        scalar2=None, op0=ALU.add)
                sqv = sb.tile([HID, 1], F32, tag="sqv")
                nc.scalar.activation(out=sqv[:], in_=vt[:], func=ACTF.Sqrt)
                rstd = sb.tile([HID, 1], F32, tag="rstd")
                nc.vector.reciprocal(out=rstd[:], in_=sqv[:])
                s_t = sb.tile([128, 1], F32, tag="sT")
                nc.vector.tensor_tensor(out=s_t[0:HID, :], in0=bng_t[l][:],
                                        in1=rstd[:], op=ALU.mult)
                t_t = sb.tile([128, 1], F32, tag="tT")
                nc.vector.tensor_tensor(out=t_t[0:HID, :], in0=mt[:],
                                        in1=s_t[0:HID, :], op=ALU.mult)
                nc.vector.tensor_tensor(out=t_t[0:HID, :], in0=bnb_t[l][:],
                                        in1=t_t[0:HID, :], op=ALU.subtract)
                nc.sync.dma_start(out=s_t[HID:128, :], in_=s_t[0:HID, :])
                nc.sync.dma_start(out=t_t[HID:128, :], in_=t_t[0:HID, :])
                nc.vector.tensor_scalar(out=h_new[:], in0=h_new[:],
                                        scalar1=s_t[:], scalar2=t_t[:],
                                        op0=ALU.mult, op1=ALU.add)
                if dbg and l == 0:
                    dh1 = drp.tile([128, NPAIR], F32, tag="dh1")
                    nc.vector.tensor_copy(out=dh1[:], in_=h_new[:])
                    nc.sync.dma_start(out=dbg_h[:], in_=dh1[:])
                h_cur = h_new

            # ---- MLP head (parity-interleaved feature-major) ----
            MB = 512
            ov = out_d[:].rearrange("c (q i) -> c q i", i=2)
            for par in range(2):
                hsec = h_cur[par * HID:(par + 1) * HID, :]
                for s0 in range(0, NPAIR, MB):
                    s1 = min(s0 + MB, NPAIR)
                    n = s1 - s0
                    p1 = psM.tile([2 * HID, MB], F32, tag="mp1", space="PSUM")
                    nc.tensor.matmul(out=p1[:, 0:n],
                                     lhsT=mw1_t[par * HID:(par + 1) * HID, :],
                                     rhs=hsec[:, s0:s1],
                                     start=True, stop=True)
                    a1 = sb2.tile([2 * HID, MB], BF16, tag="a1")
                    nc.scalar.activation(out=a1[:, 0:n], in_=p1[:, 0:n],
                                         func=ACTF.Gelu, bias=mb1_t[:])
                    p2 = psM.tile([HID, MB], F32, tag="mp2", space="PSUM")
                    nc.tensor.matmul(out=p2[:, 0:n], lhsT=mw2_t[:],
                                     rhs=a1[:, 0:n], start=True, stop=True)
                    a2 = sb2.tile([HID, MB], BF16, tag="a2")
                    nc.scalar.activation(out=a2[:, 0:n], in_=p2[:, 0:n],
                                         func=ACTF.Gelu, bias=mb2_t[:])
                    p3 = psM.tile([NCLS, MB], F32, tag="mp3", space="PSUM")
                    nc.tensor.matmul(out=p3[:, 0:n], lhsT=mw3_t[:],
                                     rhs=a2[:, 0:n], start=True, stop=True)
                    ob = sb2.tile([NCLS, MB], F32, tag="ob")
                    nc.vector.tensor_scalar(out=ob[:, 0:n], in0=p3[:, 0:n],
                                            scalar1=mb3_t[:], scalar2=None,
                                            op0=ALU.add)
                    nc.sync.dma_start(out=ov[:, s0:s1, par],
                                      in_=ob[:, 0:n])
    nc.compile()
    return nc


def kernel(x, edge_index, edge_attr,
           conv_w0, conv_b0, conv_w1, conv_b1, conv_w2, conv_b2,
           bn_g0, bn_be0, bn_g1, bn_be1, bn_g2, bn_be2,
           mlp_w1, mlp_b1, mlp_w2, mlp_b2, mlp_w3, mlp_b3):
    x = np.asarray(x)
    N, in_c = x.shape
    hid = np.asarray(conv_w0).shape[1]
    ncls = np.asarray(mlp_w3).shape[1]
    cfg = _cfg(N, in_c, hid, ncls)

    edge_index = np.asarray(edge_index)
    plan = _plan(edge_index, cfg)
    per_core = _preprocess(x, edge_index, edge_attr, cfg, plan)

    bf = ml_dtypes.bfloat16
    common = dict(
        conv_w0=np.asarray(conv_w0).astype(bf),
        conv_w1=np.asarray(conv_w1).astype(bf),
        conv_w2=np.asarray(conv_w2).astype(bf),
        conv_b0=np.asarray(conv_b0, dtype=np.float32).reshape(-1, 1),
        conv_b1=np.asarray(conv_b1, dtype=np.float32).reshape(-1, 1),
        conv_b2=np.asarray(conv_b2, dtype=np.float32).reshape(-1, 1),
        bn_g0=np.asarray(bn_g0, dtype=np.float32).reshape(-1, 1),
        bn_g1=np.asarray(bn_g1, dtype=np.float32).reshape(-1, 1),
        bn_g2=np.asarray(bn_g2, dtype=np.float32).reshape(-1, 1),
        bn_be0=np.asarray(bn_be0, dtype=np.float32).reshape(-1, 1),
        bn_be1=np.asarray(bn_be1, dtype=np.float32).reshape(-1, 1),
        bn_be2=np.asarray(bn_be2, dtype=np.float32).reshape(-1, 1),
        mlp_w1=np.asarray(mlp_w1).astype(bf),
        mlp_w2=np.asarray(mlp_w2).astype(bf),
        mlp_w3=np.asarray(mlp_w3).astype(bf),
        mlp_b1=np.asarray(mlp_b1, dtype=np.float32).reshape(-1, 1),
        mlp_b2=np.asarray(mlp_b2, dtype=np.float32).reshape(-1, 1),
        mlp_b3=np.asarray(mlp_b3, dtype=np.float32).reshape(-1, 1),
    )
    in_maps = []
    for c in range(NCORES):
        m = dict(common)
        m["x_t"] = per_core[c]["x_t"]
        m["g_idx"] = per_core[c]["g_idx"]
        m["s_idx"] = per_core[c]["s_idx"]
        m["ew_s"] = per_core[c]["ew_s"]
        in_maps.append(m)

    nc = _build(cfg, plan)
    nc.m = get_hw_module(nc.m)
    res = bass_utils.run_bass_kernel_spmd(
        nc, in_maps, core_ids=list(range(NCORES)))

    kernel._last_res = res
    kernel._last_nc = nc
    kernel._last_in_maps = in_maps
    out = np.empty((N, cfg["NCLS"]), dtype=np.float32)
    SHARD = cfg["SHARD"]
    for c in range(NCORES):
        out[c * SHARD:(c + 1) * SHARD] = res.results[c]["out5"][:, :SHARD].T
    return out


# revision 22
# speedup vs baseline: 12.4914x; 1.2530x over previous
"""Trainium2 Bass kernel for MineralDepositGCN (3x GCNConv+BN + MLP head).

Strategy (8 NeuronCores, SPMD single program), per the sharding hint:
  - Nodes sharded by range: core c owns nodes [c*12500, (c+1)*12500),
    padded to NPAD=12800. Edges owned by their DST core, so aggregation is
    device-local; the halo exchange is an AllGather of projected features.
  - Per layer: project own shard h@W into a compact node-major f32 table
    [NPAD, 64] (256B rows), AllGather -> [8*NPAD, 64] in DRAM.
  - Messages fetched with dma_gather (256B rows, all-useful f32). int16
    gather indices reach 32767 rows only, so edges are bucketed by
    src-core-pair "window" (4 windows x 25600 rows), gathered from a
    sliced table view.
  - Scatter-add via dma_scatter_add (SDMA CCE f32 accumulate) into a
    DRAM aggregate [NPAD, 64]. CCE loses concurrent duplicate updates
    within one call, so edges are split into conflict-free runs: run
    (w, k) holds each dst's k-th edge from window w; runs execute as
    separate (serialized) scatter calls. Gathers batch several runs per
    call; edge-weight scaling is one in-place broadcast multiply per
    segment. This keeps the whole edge stage at ~260 instructions/layer
    (the backend executes ~12k instructions/s, so instruction count
    dominates the runtime).
  - Aggregate drain: f32 node-major -> bf16 [6400, 128] (two nodes per
    row) -> hardware xbar dma transpose -> parity-interleaved
    feature-major h [128=(feat,parity), 6400]. Projection/BN/MLP all
    operate on this layout at unchanged instruction counts.
  - BN stats via 2 accum passes + parity fold + tiny AllReduce with
    closed-form pad-row correction.
"""
import os
import numpy as np
import ml_dtypes

from concourse import bass, bacc, tile, mybir
from concourse import bass_utils
from concourse.bass_interp import get_hw_module

BF16 = mybir.dt.bfloat16
F32 = mybir.dt.float32
I16 = mybir.dt.int16
ALU = mybir.AluOpType
ACTF = mybir.ActivationFunctionType

NCORES = 8
EPS = 1e-5
SEGMAX = 13056          # max gather-segment slots (128-mult)
SCATMAX = 8064          # max slots per dma_scatter_add call (2 desc/idx, <16384)


def _cfg(n_nodes, in_c, hid, ncls):
    shard = n_nodes // NCORES
    npad = ((shard + 511) // 512) * 512
    return dict(
        N=n_nodes, IN_C=in_c, HID=hid, NCLS=ncls,
        SHARD=shard, NPAD=npad,
        NTILES=npad // 128,
        NTOT=npad * NCORES,
        WINR=2 * npad,
        NWIN=NCORES // 2,
    )


def _plan(edge_index, cfg):
    """Conflict-free run structure shared by all cores.

    Edges keyed by (window w, rank k) where k = occurrence index of the
    edge's dst within window w on its owner core. Run (w, k) has
    RL[w][k] = 128*ceil(max_core count/128) slots. Runs are packed in
    (w, k) order into gather segments of <= SEGMAX slots.
    """
    SHARD, NWIN, NPAD = cfg["SHARD"], cfg["NWIN"], cfg["NPAD"]
    src = edge_index[0].astype(np.int64)
    dst = edge_index[1].astype(np.int64)
    d_owner = np.minimum(dst // SHARD, NCORES - 1)
    d_local = dst - d_owner * SHARD
    s_owner = np.minimum(src // SHARD, NCORES - 1)
    w = s_owner // 2

    # rank of each edge within its (owner, w, dst) group
    key = (d_owner * NWIN + w) * SHARD + d_local
    order = np.argsort(key, kind="stable")
    sk = key[order]
    new = np.ones(len(sk), dtype=bool)
    new[1:] = sk[1:] != sk[:-1]
    idxs = np.arange(len(sk))
    starts = idxs[new]
    grp_start = starts[np.cumsum(new) - 1]
    rank_sorted = idxs - grp_start
    rank = np.empty(len(sk), dtype=np.int64)
    rank[order] = rank_sorted

    KMAX = int(rank.max()) + 1
    cnt = np.zeros((NCORES, NWIN, KMAX), dtype=np.int64)
    np.add.at(cnt.reshape(-1), (d_owner * NWIN + w) * KMAX + rank, 1)
    cmax = cnt.max(axis=0)                       # [NWIN, KMAX]
    RL = ((cmax + 127) // 128) * 128
    run_off = np.zeros((NWIN, KMAX), dtype=np.int64)
    segments = []                                # per window
    off = 0
    for wi in range(NWIN):
        segs = []
        cur_off, cur_n, cur_runs = off, 0, []
        for k in range(KMAX):
            rl = int(RL[wi, k])
            if rl == 0:
                continue
            if cur_n + rl > SEGMAX and cur_n > 0:
                segs.append((cur_off, cur_n, cur_runs))
                cur_off, cur_n, cur_runs = off, 0, []
            run_off[wi, k] = off
            cur_runs.append(k)
            cur_n += rl
            off += rl
        if cur_n > 0:
            segs.append((cur_off, cur_n, cur_runs))
        segments.append(segs)
    NSLOT = int(off)
    return dict(rank=rank, w=w, d_owner=d_owner, d_local=d_local,
                RL=RL, run_off=run_off, segments=segments,
                NSLOT=NSLOT, KMAX=KMAX)


def _preprocess(x, edge_index, edge_attr, cfg, plan):
    SHARD, NPAD, NWIN = cfg["SHARD"], cfg["NPAD"], cfg["NWIN"]
    WINR = cfg["WINR"]
    NSLOT = plan["NSLOT"]
    src = edge_index[0].astype(np.int64)
    ew = np.asarray(edge_attr, dtype=np.float32)
    s_owner = np.minimum(src // SHARD, NCORES - 1)
    s_local = src - s_owner * SHARD
    gidx_all = (s_owner * NPAD + s_local) - plan["w"] * WINR
    AGGR = NPAD + 128
    JUNK = NPAD                                  # scatter junk row

    bf = ml_dtypes.bfloat16
    per_core = []
    for c in range(NCORES):
        m = plan["d_owner"] == c
        wi = plan["w"][m]
        k = plan["rank"][m]
        gi = gidx_all[m]
        dl = plan["d_local"][m]
        we = ew[m]
        # position within run: order by (w, k, dst)
        order = np.lexsort((dl, k, wi))
        wi, k, gi, dl, we = (wi[order], k[order], gi[order], dl[order],
                             we[order])
        runkey = wi * plan["KMAX"] + k
        new = np.ones(len(runkey), dtype=bool)
        new[1:] = runkey[1:] != runkey[:-1]
        idxs = np.arange(len(runkey))
        starts = idxs[new]
        pos = idxs - starts[np.cumsum(new) - 1]
        slot = plan["run_off"][wi, k] + pos

        gidx16 = np.zeros(NSLOT, dtype=np.int16)
        sidx16 = np.full(NSLOT, JUNK, dtype=np.int16)
        ewb = np.zeros(NSLOT, dtype=np.float32)
        gidx16[slot] = gi.astype(np.int16)
        sidx16[slot] = (dl + (k % 2) * AGGR).astype(np.int16)
        ewb[slot] = we

        # combined per-segment blocks: [gather idx | scatter idx]
        cidx = np.zeros(2 * NSLOT, dtype=np.int16)
        for segs in plan["segments"]:
            for (soff, nsl, _runs) in segs:
                cidx[2 * soff:2 * soff + nsl] = gidx16[soff:soff + nsl]
                cidx[2 * soff + nsl:2 * (soff + nsl)] = \
                    sidx16[soff:soff + nsl]

        per_core.append(dict(
            c_idx=cidx.reshape(2 * NSLOT // 16, 16).T.copy(),
            ew_s=ewb.reshape(NSLOT // 128, 128).T.astype(bf),
        ))

    for c in range(NCORES):
        xs = np.zeros((cfg["IN_C"], NPAD), dtype=np.float32)
        xs[:, :SHARD] = np.asarray(x[c * SHARD:(c + 1) * SHARD]).T
        per_core[c]["x_t"] = xs.astype(bf)
    return per_core


def _build(cfg, plan):
    IN_C, HID, NCLS = cfg["IN_C"], cfg["HID"], cfg["NCLS"]
    NPAD, NTILES = cfg["NPAD"], cfg["NTILES"]
    NTOT, WINR, NWIN = cfg["NTOT"], cfg["WINR"], cfg["NWIN"]
    NSLOT = plan["NSLOT"]
    RL, run_off, segments = plan["RL"], plan["run_off"], plan["segments"]
    NPAIR = NPAD // 2
    AGGR = NPAD + 128
    NPADDING = float(NCORES * NPAD - cfg["N"])
    INVN = 1.0 / cfg["N"]
    SEGC = SEGMAX // 128
    LVL = int(os.environ.get("KLEVEL", "0"))
    NCH = NSLOT // 128

    nc = bacc.Bacc("TRN2", target_bir_lowering=False, debug=False,
                   num_devices=NCORES)

    def din(name, shape, dt):
        return nc.dram_tensor(name, shape, dt, kind="ExternalInput").ap()

    x_t_d = din("x_t", [IN_C, NPAD], BF16)
    cidx_d = din("c_idx", [16, 2 * NSLOT // 16], I16)
    ew_d = din("ew_s", [128, NCH], BF16)
    pcorr_d = din("pad_corr", [HID, 6], F32)
    cw_d = [din(f"conv_w{l}", [IN_C if l == 0 else HID, HID], BF16)
            for l in range(3)]
    cb_d = [din(f"conv_b{l}", [HID, 1], F32) for l in range(3)]
    bng_d = [din(f"bn_g{l}", [HID, 1], F32) for l in range(3)]
    bnb_d = [din(f"bn_be{l}", [HID, 1], F32) for l in range(3)]
    mw1_d = din("mlp_w1", [HID, 2 * HID], BF16)
    mw2_d = din("mlp_w2", [2 * HID, HID], BF16)
    mw3_d = din("mlp_w3", [HID, NCLS], BF16)
    mb1_d = din("mlp_b1", [2 * HID, 1], F32)
    mb2_d = din("mlp_b2", [HID, 1], F32)
    mb3_d = din("mlp_b3", [NCLS, 1], F32)
    out_d = nc.dram_tensor("out5", [NCLS, NPAD], F32,
                           kind="ExternalOutput").ap()
    dbg = bool(os.environ.get("KDBG"))
    if dbg:
        dbg_tab = nc.dram_tensor("dbg_tab", [NTOT, 64], F32,
                                 kind="ExternalOutput").ap()
        dbg_agg = nc.dram_tensor("dbg_agg", [NPAD, 64], F32,
                                 kind="ExternalOutput").ap()
        dbg_h = nc.dram_tensor("dbg_h", [128, NPAIR], F32,
                               kind="ExternalOutput").ap()

    rg = [list(range(NCORES))]

    with tile.TileContext(nc) as tc:
        with tc.tile_pool(name="sb", bufs=1) as sb, \
             tc.tile_pool(name="sb2", bufs=2) as sb2, \
             tc.tile_pool(name="hxp", bufs=2) as hxp, \
             tc.tile_pool(name="drp", bufs=1) as drp, \
             tc.tile_pool(name="mp", bufs=2) as mp, \
             tc.tile_pool(name="idxp", bufs=2) as idxp, \
             tc.tile_pool(name="psP", bufs=2, space="PSUM") as psP, \
             tc.tile_pool(name="psM", bufs=1, space="PSUM") as psM, \
             tc.tile_pool(name="dram", bufs=1, space="DRAM") as dram, \
             tc.tile_pool(name="dram2", bufs=2, space="DRAM") as dram2:

            # ---- persistent loads ----
            ew_f = sb.tile([128, NCH], F32, tag="ewf")
            EWCH = 1024
            for e0 in range(0, NCH, EWCH):
                e1 = min(e0 + EWCH, NCH)
                ewt = sb2.tile([128, EWCH], BF16, tag="ewtmp")
                nc.sync.dma_start(out=ewt[:, 0:e1 - e0], in_=ew_d[:, e0:e1])
                nc.vector.tensor_copy(out=ew_f[:, e0:e1],
                                      in_=ewt[:, 0:e1 - e0])
            cw_t = []
            for l in range(3):
                if l == 0:
                    t = sb.tile([IN_C, HID], BF16, tag=f"cw{l}")
                    nc.sync.dma_start(out=t[:], in_=cw_d[l][:])
                else:
                    # duplicated across parity halves so lhsT/rhs base
                    # partitions match for the odd-node projection
                    t = sb.tile([128, HID], BF16, tag=f"cw{l}")
                    nc.sync.dma_start(out=t[0:HID, :], in_=cw_d[l][:])
                    nc.sync.dma_start(out=t[HID:128, :], in_=cw_d[l][:])
                cw_t.append(t)
            pcorr_t = sb.tile([HID, 6], F32, tag="pcorr")
            nc.sync.dma_start(out=pcorr_t[:], in_=pcorr_d[:])
            eps_t = sb.tile([HID, 1], F32, tag="eps")
            nc.vector.memset(eps_t[:], EPS)
            bng_t, bnb_t, cb2_t = [], [], []
            for l in range(3):
                tb2 = sb.tile([128, 1], F32, tag=f"cbd{l}")
                nc.sync.dma_start(out=tb2[0:HID, :], in_=cb_d[l][:])
                nc.sync.dma_start(out=tb2[HID:128, :], in_=cb_d[l][:])
                cb2_t.append(tb2)
                tg = sb.tile([HID, 1], F32, tag=f"bng{l}")
                nc.sync.dma_start(out=tg[:], in_=bng_d[l][:])
                bng_t.append(tg)
                te = sb.tile([HID, 1], F32, tag=f"bnb{l}")
                nc.sync.dma_start(out=te[:], in_=bnb_d[l][:])
                bnb_t.append(te)
            mw1_t = sb.tile([128, 2 * HID], BF16, tag="mw1")
            nc.sync.dma_start(out=mw1_t[0:HID, :], in_=mw1_d[:])
            nc.sync.dma_start(out=mw1_t[HID:128, :], in_=mw1_d[:])
            mw2_t = sb.tile([2 * HID, HID], BF16, tag="mw2")
            nc.sync.dma_start(out=mw2_t[:], in_=mw2_d[:])
            mw3_t = sb.tile([HID, NCLS], BF16, tag="mw3")
            nc.sync.dma_start(out=mw3_t[:], in_=mw3_d[:])
            mb1_t = sb.tile([2 * HID, 1], F32, tag="mb1")
            nc.sync.dma_start(out=mb1_t[:], in_=mb1_d[:])
            mb2_t = sb.tile([HID, 1], F32, tag="mb2")
            nc.sync.dma_start(out=mb2_t[:], in_=mb2_d[:])
            mb3_t = sb.tile([NCLS, 1], F32, tag="mb3")
            nc.sync.dma_start(out=mb3_t[:], in_=mb3_d[:])
            zt = sb.tile([128, 1664], F32, tag="zero")
            nc.vector.memset(zt[:], 0.0)

            x_t = sb.tile([IN_C, NPAD], BF16, tag="hx0")
            nc.sync.dma_start(out=x_t[:], in_=x_t_d[:])

            # replicate wrapped indices [16, X] -> [128, X] in DRAM
            cidx_rep = dram.tile([128, 2 * NSLOT // 16], I16, tag="cidxrep")
            if LVL < 3:
                for r in range(8):
                    nc.sync.dma_start(
                        out=cidx_rep[16 * r:16 * (r + 1), :], in_=cidx_d[:])
            # pre-zeroed DRAM block to reset the aggregate each layer
            zdram = dram.tile([2 * AGGR, 64], F32, tag="zdram")
            zv = zdram[:].rearrange("(k p) f -> p k f", p=128)
            NZCH = 2 * AGGR // 128
            ZC = 1664 // 64
            for z0 in range(0, NZCH, ZC):
                z1 = min(z0 + ZC, NZCH)
                nc.sync.dma_start(
                    out=zv[:, z0:z1, :],
                    in_=zt[:, 0:(z1 - z0) * 64].rearrange(
                        "p (k f) -> p k f", f=64))

            h_cur = x_t              # layer0: plain feature-major
            for l in range(3):
                cdim = IN_C if l == 0 else HID
                # ---- projection -> compact node-major f32 table ----
                tab_in = dram2.tile([NPAD, 64], F32, tag="tabin")
                PB = 16
                if l == 0:
                    srcs = [(h_cur, 0, NTILES, tab_in[:])]
                else:
                    evens = tab_in[:].rearrange("(q i) f -> q i f", i=2)
                    srcs = [(h_cur, 0, NTILES // 2, evens[:, 0, :]),
                            (h_cur, HID, NTILES // 2, evens[:, 1, :])]
                for hsrc, prow, ntile, tview in srcs:
                    for g in range(0, ntile, PB):
                        gn = min(PB, ntile - g)
                        pp = psP.tile([128, PB * HID], F32, tag="proj",
                                      space="PSUM")
                        for j in range(gn):
                            kk = g + j
                            nc.tensor.matmul(
                                out=pp[:, j * HID:(j + 1) * HID],
                                lhsT=hsrc[prow:prow + cdim,
                                          kk * 128:(kk + 1) * 128],
                                rhs=cw_t[l][prow:prow + cdim, :],
                                start=True, stop=True)
                        stg = sb2.tile([128, PB, HID], F32, tag="stage")
                        nc.any.tensor_copy(
                            out=stg[:, 0:gn, :],
                            in_=pp[:, 0:gn * HID].rearrange(
                                "p (j f) -> p j f", j=gn))
                        tv = tview.rearrange("(g p) f -> g p f", p=128)
                        nc.sync.dma_start(
                            out=tv[g:g + gn].rearrange("g p f -> p g f"),
                            in_=stg[:, 0:gn, :])

                # ---- AllGather the projected table ----
                tab_full = dram2.tile([NTOT, 64], F32, tag="tabfull")
                if LVL < 4:
                    nc.gpsimd.collective_compute(
                        "AllGather", ALU.bypass, replica_groups=rg,
                        ins=[tab_in[:]], outs=[tab_full[:]])

                # ---- zero aggregate, gather + scatter-add ----
                agg = dram2.tile([2 * AGGR, 64], F32, tag="agg")
                nc.sync.dma_start(out=agg[:], in_=zdram[:])
                for wi in range(NWIN):
                    for (soff, nsl, runs) in segments[wi]:
                        segc = nsl // 128
                        if LVL < 3:
                            ci_t = idxp.tile([128, 2 * SEGC * 8], I16,
                                             tag="ci")
                            nc.sync.dma_start(
                                out=ci_t[:, 0:2 * nsl // 16],
                                in_=cidx_rep[:, 2 * soff // 16:
                                             2 * (soff + nsl) // 16])
                            gi_t = ci_t[:, 0:nsl // 16]
                            si_t = ci_t[:, nsl // 16:2 * nsl // 16]
                            m_t = mp.tile([128, SEGC, 64], F32, tag="mt")
                            nc.gpsimd.dma_gather(
                                out_ap=m_t[:, 0:segc, :],
                                in_ap=tab_full[wi * WINR:(wi + 1) * WINR, :],
                                idxs_ap=gi_t,
                                num_idxs=nsl, num_idxs_reg=nsl,
                                elem_size=64, single_packet=False)
                        if LVL < 2:
                            a0, a1 = bass.broadcast_tensor_aps(
                                m_t[:, 0:segc, :],
                                ew_f[:, soff // 128:soff // 128 + segc]
                                .rearrange("p (k a) -> p k a", a=1))
                            nc.vector.tensor_tensor(
                                out=m_t[:, 0:segc, :], in0=a0, in1=a1,
                                op=ALU.mult)
                        if LVL < 1:
                            # pair-merged conflict-free spans (k//2 groups)
                            spans = []
                            for k in runs:
                                ro, rl = int(run_off[wi][k]), int(RL[wi][k])
                                if spans and spans[-1][2] == k // 2 and \
                                        spans[-1][1] == ro:
                                    spans[-1] = (spans[-1][0], ro + rl,
                                                 k // 2)
                                else:
                                    spans.append((ro, ro + rl, k // 2))
                            for (a, b, _pid) in spans:
                                for p0 in range(a, b, SCATMAX):
                                    pn = min(SCATMAX, b - p0)
                                    lo = p0 - soff
                                    nc.gpsimd.dma_scatter_add(
                                        out_ap=agg[:],
                                        in_ap=m_t[:, lo // 128:
                                                  (lo + pn) // 128, :],
                                        idxs_ap=si_t[:, lo // 16:
                                                     (lo + pn) // 16],
                                        num_idxs=pn, num_idxs_reg=pn,
                                        elem_size=64, single_packet=False)

                # ---- drain: f32 node-major -> bf16 pair-rows -> xbar T ----
                if dbg and l == 0:
                    nc.sync.dma_start(out=dbg_tab[:], in_=tab_full[:])
                    nc.sync.dma_start(out=dbg_agg[:], in_=agg[0:NPAD, :])  # even half only
                af = drp.tile([128, NTILES, 64], F32, tag="af")
                av0 = agg[0:NPAD, :].rearrange("(k p) f -> p k f", p=128)
                av1 = agg[AGGR:AGGR + NPAD, :].rearrange(
                    "(k p) f -> p k f", p=128)
                nc.sync.dma_start(out=af[:], in_=av0)
                HT = NTILES // 2
                for hh in range(2):
                    nc.gpsimd.dma_start(
                        out=af[:, hh * HT:(hh + 1) * HT, :],
                        in_=av1[:, hh * HT:(hh + 1) * HT, :],
                        accum_op=ALU.add)
                abf = drp.tile([128, NTILES, 64], BF16, tag="abf")
                nc.vector.tensor_copy(out=abf[:], in_=af[:])
                tmp = dram2.tile([NPAIR, 128], BF16, tag="ptmp")
                nc.sync.dma_start(
                    out=tmp[:].rearrange("(k q) (i f) -> (q i) k f",
                                         q=64, i=2),
                    in_=abf[:])
                h_new = hxp.tile([128, NPAIR], BF16, tag="hx")
                nc.sync.dma_start_transpose(out=h_new[:], in_=tmp[:])
                # bias + relu (bias duplicated across parity halves)
                nc.vector.tensor_scalar(
                    out=h_new[:], in0=h_new[:], scalar1=cb2_t[l][:],
                    scalar2=0.0, op0=ALU.add, op1=ALU.max)

                # ---- BN stats (global, pad-row corrected) ----
                scr = drp.tile([128, NPAIR], BF16, tag="scr")
                st2 = sb.tile([128, 2], F32, tag="st2")
                nc.scalar.activation(out=scr[:], in_=h_new[:],
                                     func=ACTF.Copy, accum_out=st2[:, 0:1])
                nc.scalar.activation(out=scr[:], in_=h_new[:],
                                     func=ACTF.Square, accum_out=st2[:, 1:2])
                fold = sb.tile([HID, 2], F32, tag="fold")
                nc.sync.dma_start(out=fold[:], in_=st2[HID:128, :])
                sc = sb.tile([HID, 2], F32, tag="statsc")
                nc.vector.tensor_tensor(out=sc[:], in0=pcorr_t[:, 2 * l:
                                                              2 * l + 2],
                                        in1=st2[0:HID, :], op=ALU.add)
                nc.vector.tensor_tensor(out=sc[:], in0=sc[:],
                                        in1=fold[:], op=ALU.add)
                stats_in = dram.tile([HID, 2], F32, tag=f"stin{l}")
                stats_out = dram.tile([HID, 2], F32, tag=f"stout{l}")
                st_sb = sb.tile([HID, 2], F32, tag="stsb")
                if LVL < 4:
                    nc.sync.dma_start(out=stats_in[:], in_=sc[:])
                    nc.gpsimd.collective_compute(
                        "AllReduce", ALU.add, replica_groups=rg,
                        ins=[stats_in[:]], outs=[stats_out[:]])
                    nc.sync.dma_start(out=st_sb[:], in_=stats_out[:])
                else:
                    nc.vector.tensor_copy(out=st_sb[:], in_=sc[:])
                mt = sb.tile([HID, 1], F32, tag="mt1")
                nc.vector.tensor_scalar(out=mt[:], in0=st_sb[:, 0:1],
                                        scalar1=INVN, scalar2=None,
                                        op0=ALU.mult)
                vt = sb.tile([HID, 1], F32, tag="vt")
                nc.vector.tensor_scalar(out=vt[:], in0=st_sb[:, 1:2],
                                        scalar1=INVN, scalar2=None,
                                        op0=ALU.mult)
                msq = sb.tile([HID, 1], F32, tag="msq")
                nc.vector.tensor_tensor(out=msq[:], in0=mt[:], in1=mt[:],
                                        op=ALU.mult)
                nc.vector.tensor_tensor(out=vt[:], in0=vt[:], in1=msq[:],
                                        op=ALU.subtract)
                sqv = sb.tile([HID, 1], F32, tag="sqv")
                nc.scalar.activation(out=sqv[:], in_=vt[:], func=ACTF.Sqrt,
                                     bias=eps_t[:])
                rstd = sb.tile([HID, 1], F32, tag="rstd")
                nc.vector.reciprocal(out=rstd[:], in_=sqv[:])
                s_t = sb.tile([128, 1], F32, tag="sT")
                nc.vector.tensor_tensor(out=s_t[0:HID, :], in0=bng_t[l][:],
                                        in1=rstd[:], op=ALU.mult)
                t_t = sb.tile([128, 1], F32, tag="tT")
                nc.vector.tensor_tensor(out=t_t[0:HID, :], in0=mt[:],
                                        in1=s_t[0:HID, :], op=ALU.mult)
                nc.vector.tensor_tensor(out=t_t[0:HID, :], in0=bnb_t[l][:],
                                        in1=t_t[0:HID, :], op=ALU.subtract)
                nc.sync.dma_start(out=s_t[HID:128, :], in_=s_t[0:HID, :])
                nc.sync.dma_start(out=t_t[HID:128, :], in_=t_t[0:HID, :])
                nc.vector.tensor_scalar(out=h_new[:], in0=h_new[:],
                                        scalar1=s_t[:], scalar2=t_t[:],
                                        op0=ALU.mult, op1=ALU.add)
                if dbg and l == 0:
                    dh1 = drp.tile([128, NPAIR], F32, tag="dh1")
                    nc.vector.tensor_copy(out=dh1[:], in_=h_new[:])
                    nc.sync.dma_start(out=dbg_h[:], in_=dh1[:])
                h_cur = h_new

            # ---- MLP head (parity-interleaved feature-major) ----
            MB = 512
            ov = out_d[:].rearrange("c (q i) -> c q i", i=2)
            for par in range(2):
                hsec = h_cur[par * HID:(par + 1) * HID, :]
                for s0 in range(0, NPAIR, MB):
                    s1 = min(s0 + MB, NPAIR)
                    n = s1 - s0
                    p1 = psM.tile([2 * HID, MB], F32, tag="mp1", space="PSUM")
                    nc.tensor.matmul(out=p1[:, 0:n],
                                     lhsT=mw1_t[par * HID:(par + 1) * HID, :],
                                     rhs=hsec[:, s0:s1],
                                     start=True, stop=True)
                    a1 = sb2.tile([2 * HID, MB], BF16, tag="a1")
                    nc.scalar.activation(out=a1[:, 0:n], in_=p1[:, 0:n],
                                         func=ACTF.Gelu, bias=mb1_t[:])
                    p2 = psM.tile([HID, MB], F32, tag="mp2", space="PSUM")
                    nc.tensor.matmul(out=p2[:, 0:n], lhsT=mw2_t[:],
                                     rhs=a1[:, 0:n], start=True, stop=True)
                    a2 = sb2.tile([HID, MB], BF16, tag="a2")
                    nc.scalar.activation(out=a2[:, 0:n], in_=p2[:, 0:n],
                                         func=ACTF.Gelu, bias=mb2_t[:])
                    p3 = psM.tile([NCLS, MB], F32, tag="mp3", space="PSUM")
                    nc.tensor.matmul(out=p3[:, 0:n], lhsT=mw3_t[:],
                                     rhs=a2[:, 0:n], start=True, stop=True)
                    ob = sb2.tile([NCLS, MB], F32, tag="ob")
                    nc.vector.tensor_scalar(out=ob[:, 0:n], in0=p3[:, 0:n],
                                            scalar1=mb3_t[:], scalar2=None,
                                            op0=ALU.add)
                    nc.sync.dma_start(out=ov[:, s0:s1, par],
                                      in_=ob[:, 0:n])
    nc.compile()
    return nc


def kernel(x, edge_index, edge_attr,
           conv_w0, conv_b0, conv_w1, conv_b1, conv_w2, conv_b2,
           bn_g0, bn_be0, bn_g1, bn_be1, bn_g2, bn_be2,
           mlp_w1, mlp_b1, mlp_w2, mlp_b2, mlp_w3, mlp_b3):
    x = np.asarray(x)
    N, in_c = x.shape
    hid = np.asarray(conv_w0).shape[1]
    ncls = np.asarray(mlp_w3).shape[1]
    cfg = _cfg(N, in_c, hid, ncls)

    edge_index = np.asarray(edge_index)
    plan = _plan(edge_index, cfg)
    per_core = _preprocess(x, edge_index, edge_attr, cfg, plan)

    bf = ml_dtypes.bfloat16
    common = dict(
        conv_w0=np.asarray(conv_w0).astype(bf),
        conv_w1=np.asarray(conv_w1).astype(bf),
        conv_w2=np.asarray(conv_w2).astype(bf),
        conv_b0=np.asarray(conv_b0, dtype=np.float32).reshape(-1, 1),
        conv_b1=np.asarray(conv_b1, dtype=np.float32).reshape(-1, 1),
        conv_b2=np.asarray(conv_b2, dtype=np.float32).reshape(-1, 1),
        bn_g0=np.asarray(bn_g0, dtype=np.float32).reshape(-1, 1),
        bn_g1=np.asarray(bn_g1, dtype=np.float32).reshape(-1, 1),
        bn_g2=np.asarray(bn_g2, dtype=np.float32).reshape(-1, 1),
        bn_be0=np.asarray(bn_be0, dtype=np.float32).reshape(-1, 1),
        bn_be1=np.asarray(bn_be1, dtype=np.float32).reshape(-1, 1),
        bn_be2=np.asarray(bn_be2, dtype=np.float32).reshape(-1, 1),
        mlp_w1=np.asarray(mlp_w1).astype(bf),
        mlp_w2=np.asarray(mlp_w2).astype(bf),
        mlp_w3=np.asarray(mlp_w3).astype(bf),
        mlp_b1=np.asarray(mlp_b1, dtype=np.float32).reshape(-1, 1),
        mlp_b2=np.asarray(mlp_b2, dtype=np.float32).reshape(-1, 1),
        mlp_b3=np.asarray(mlp_b3, dtype=np.float32).reshape(-1, 1),
    )
    npadding = float(NCORES * cfg["NPAD"] - N)
    pc = np.zeros((hid, 6), np.float32)
    for l, cb in enumerate([conv_b0, conv_b1, conv_b2]):
        pb = np.maximum(np.asarray(cb, dtype=np.float32), 0.0)
        pc[:, 2 * l] = -npadding * pb
        pc[:, 2 * l + 1] = -npadding * pb * pb
    common["pad_corr"] = pc
    in_maps = []
    for c in range(NCORES):
        m = dict(common)
        m["x_t"] = per_core[c]["x_t"]
        m["c_idx"] = per_core[c]["c_idx"]
        m["ew_s"] = per_core[c]["ew_s"]
        in_maps.append(m)

    nc = _build(cfg, plan)
    nc.m = get_hw_module(nc.m)
    res = bass_utils.run_bass_kernel_spmd(
        nc, in_maps, core_ids=list(range(NCORES)))

    kernel._last_res = res
    kernel._last_nc = nc
    kernel._last_in_maps = in_maps
    out = np.empty((N, cfg["NCLS"]), dtype=np.float32)
    SHARD = cfg["SHARD"]
    for c in range(NCORES):
        out[c * SHARD:(c + 1) * SHARD] = res.results[c]["out5"][:, :SHARD].T
    return out
